# revision 1
# baseline (speedup 1.0000x reference)
"""Trainium2 Bass kernel for diffusers AttnProcessor self-attention.

Reference computation (fp32, B=2, S=4096, C=512, H=8, D=64):
    q = hs @ Wq.T ; k = hs @ Wk.T ; v = hs @ Wv.T          (per-head split)
    probs = softmax(q k^T / sqrt(D))                        [b,h,s,s]
    out = (probs @ v) @ Wo.T + bo                           [b,s,c]

Sharding: 8 cores = (batch b in 0..1) x (query-slice of 1024 rows in 0..3).
Each core holds the full X[b] (for K/V projections) and produces the full
output rows for its query slice -> the host just concatenates (no host math
beyond layout prep of the inputs).

Device dataflow per core (all matmuls bf16 in / fp32 PSUM accum):
  Xt = X[b]^T in SBUF                              [C=512, S=4096]
  Qt = (Wq^T/sqrt(D)) @ Xt_q  per head-pair        [128, 1024]
  Kt = Wk^T @ Xt              per head-pair        [128, 4096]
  (a per-head copy of Qt/Kt rows is DMA'd to the opposite partition half so
   the two sq-chunks of the QK^T matmul run in disjoint PE row groups)
  V' = [X @ Wv^T | 1] per head                     [S, 65] per head
  per head h, per key tile t (128 keys):
    St[t] = Kt_h[:,t]^T Qt_h        [128 sk, 1024 sq]  (2 row-packed matmuls)
    Pt    = exp(St)                 (ScalarE, bf16 out)
    O'_h += V'[t]^T Pt              [65, 1024]  (row 64 = softmax denominator)
  O_h = O'_h[0:64] * (1/O'_h[64])   -> Ot (head-concat layout)
  out = Ot^T @ Wo^T + bo            -> DMA out  [1024, 512] fp32
"""

import numpy as np
import ml_dtypes
from contextlib import ExitStack

import concourse.bass as bass
import concourse.bacc as bacc
import concourse.mybir as mybir
import concourse.tile as tile
from concourse.bass_utils import run_bass_kernel_spmd
from concourse import dve_ops as _dve_ops
from concourse.dve_spec import (
    Spec as _Spec, Src0 as _Src0, C0 as _C0, C1 as _C1, C2 as _C2,
    sq as _sq, lower as _dve_lower, _has_src1,
)
from concourse.dve_uop import DveOpSpec as _DveOpSpec

BF16 = mybir.dt.bfloat16
F32 = mybir.dt.float32

B, S, C, H, D = 2, 4096, 512, 8, 64
NCORES = 8
SQ = 1024          # query rows per core
P = 128            # partitions
NSK = S // P       # 32 key tiles
NCI = C // P       # 4 contraction tiles for projections
SQC = 512          # matmul moving free dim
NSQC = SQ // SQC   # 2
E = D + 1          # V' columns per head (64 v cols + ones col)

ROW_PACK = True    # run the two K=64 QK^T matmuls in disjoint PE row groups
DVE_EXP = False    # offload every 4th exp tile from ScalarE to a custom DVE op

# quadratic Chebyshev fit of exp(x/16) on [-2.2, 2.2]; q(x)^16 ~ exp(x)
# (max rel err 0.2% in range; scores here are < +-1.3)
_EXPC = (1.0, 0.06264781, 0.00195543)


def _register_exp16():
    """Register a custom DVE op computing q(x)^16 ~ exp(x) (8 ALU stages)."""
    for op in _dve_ops.OPS:
        if op.name == "EXP16_ANT":
            return op
    q = (_Src0 * _C2 + _C1) * _Src0 + _C0
    spec = _Spec(
        body=_sq(_sq(_sq(_sq(q)))),
        reference=lambda in0, in1, s0, s1, imm2: (
            ((in0 * np.float32(imm2) + np.float32(s1)) * in0 + np.float32(s0))
            ** 16).astype(np.float32),
    )
    idx = max(_dve_ops._SUB_OPCODE_FOR_NAME.values()) + 1
    assert idx < 0x20
    op = _dve_ops.DveOp("EXP16_ANT", spec, subdim=False, uops_sha={})
    _dve_ops.OPS.append(op)
    _dve_ops.CUSTOM_DVE_SPECS[op.name] = spec
    _dve_ops._SUB_OPCODE_FOR_NAME[op.name] = idx
    for ver in ("v3",):
        s = _DveOpSpec(name=op.name, opcode=idx, uops=_dve_lower(spec, ver=ver),
                       rd1_en=_has_src1(spec))
        op.uops_sha[ver] = s.sha(ver)
    return op


EXP16 = _register_exp16()


def build_nc(row_pack=ROW_PACK, reps=1, dve_exp=None):
    if dve_exp is None:
        dve_exp = DVE_EXP
    nc = bacc.Bacc("TRN2", target_bir_lowering=False, debug=False,
                   num_devices=NCORES)

    xt_d = nc.dram_tensor("xt", [C, S], BF16, kind="ExternalInput").ap()
    xtq_d = nc.dram_tensor("xtq", [C, SQ], BF16, kind="ExternalInput").ap()
    wqt_d = nc.dram_tensor("wqt", [C, C], BF16, kind="ExternalInput").ap()
    wkt_d = nc.dram_tensor("wkt", [C, C], BF16, kind="ExternalInput").ap()
    wvt_d = nc.dram_tensor("wvt", [C, C], BF16, kind="ExternalInput").ap()
    wot_d = nc.dram_tensor("wot", [C, C], BF16, kind="ExternalInput").ap()
    bob_d = nc.dram_tensor("bob", [P, C], F32, kind="ExternalInput").ap()
    out_d = nc.dram_tensor("out", [SQ, C], F32, kind="ExternalOutput").ap()

    with ExitStack() as ctx:
        tc = ctx.enter_context(tile.TileContext(nc))
        const = ctx.enter_context(tc.tile_pool(name="const", bufs=1))
        work = ctx.enter_context(tc.tile_pool(name="work", bufs=2))
        psum = ctx.enter_context(tc.tile_pool(name="psum", bufs=2, space="PSUM"))

        def load_tiles(dram_ap, n, cols, dtype, base, eng=None):
            tiles = []
            for ci in range(n):
                t = const.tile([P, cols], dtype, name=f"{base}{ci}",
                               tag=f"{base}{ci}")
                (eng or nc.sync).dma_start(t, dram_ap[ci * P:(ci + 1) * P, :])
                tiles.append(t)
            return tiles

        # Input loads split between the SP queue and the (startup-idle) ACT
        # queue, ordered by first use; the first QK^T tile needs
        # xtq+wqt+wkt+xt[ck0] only. Dependent SBUF<->SBUF moves go on the
        # gpsimd queue so they can't FIFO-block behind these.
        xtq_sb = load_tiles(xtq_d, NCI, SQ, BF16, "xtqs", eng=nc.scalar)
        wqt_sb = load_tiles(wqt_d, NCI, C, BF16, "wqts", eng=nc.scalar)
        wkt_sb = load_tiles(wkt_d, NCI, C, BF16, "wkts")
        xt_sb = [const.tile([P, S], BF16, name=f"xts{ci}", tag=f"xts{ci}")
                 for ci in range(NCI)]
        for ci in range(NCI):
            nc.sync.dma_start(xt_sb[ci][:, 0:SQC], xt_d[ci * P:(ci + 1) * P, 0:SQC])
        wvt_sb = load_tiles(wvt_d, NCI, C, BF16, "wvts")
        for ck in range(1, S // SQC):
            for ci in range(NCI):
                sl = slice(ck * SQC, (ck + 1) * SQC)
                nc.sync.dma_start(xt_sb[ci][:, sl], xt_d[ci * P:(ci + 1) * P, sl])
        wot_sb = load_tiles(wot_d, NCI, C, BF16, "wots")
        bob_sb = const.tile([P, C], F32, name="bobs", tag="bobs")
        nc.sync.dma_start(bob_sb, bob_d)
        ones_sb = const.tile([P, D], mybir.dt.float16, name="ones_sb",
                             tag="ones_sb")
        nc.vector.memset(ones_sb, 1.0)

        for rep in range(reps):
            emit_body(nc, tc, const, work, psum, (row_pack, dve_exp),
                      xt_sb, xtq_sb, wqt_sb, wkt_sb, wvt_sb, wot_sb,
                      bob_sb, ones_sb, out_d)

    nc.compile()
    return nc


def emit_body(nc, tc, const, work, psum, flags,
              xt_sb, xtq_sb, wqt_sb, wkt_sb, wvt_sb, wot_sb,
              bob_sb, ones_sb, out_d):
    row_pack, dve_exp = flags
    vp_sb = [None] * NSK

    def emit_vproj(t_i):
        vps = psum.tile([P, C], F32, name="vps", tag="proj")
        for ci in range(NCI):
            nc.tensor.matmul(vps, lhsT=xt_sb[ci][:, t_i * P:(t_i + 1) * P],
                             rhs=wvt_sb[ci],
                             start=(ci == 0), stop=(ci == NCI - 1))
        vp = const.tile([P, H * E], BF16, name=f"vp{t_i}", tag=f"vp{t_i}")
        vp3 = vp.rearrange("p (h e) -> p h e", e=E)
        nc.vector.tensor_copy(out=vp3[:, :, 0:D],
                              in_=vps.rearrange("p (h d) -> p h d", d=D))
        nc.vector.memset(vp3[:, :, D:E], 1.0)
        vp_sb[t_i] = vp

    def emit_qtp(p):
        qtp = work.tile([P, SQ], BF16, name="qtp", tag="qtp")
        for cq in range(NSQC):
            qps = psum.tile([P, SQC], F32, name="qps", tag="proj")
            for ci in range(NCI):
                nc.tensor.matmul(
                    qps, lhsT=wqt_sb[ci][:, p * P:(p + 1) * P],
                    rhs=xtq_sb[ci][:, cq * SQC:(cq + 1) * SQC],
                    start=(ci == 0), stop=(ci == NCI - 1))
            nc.vector.tensor_copy(out=qtp[:, cq * SQC:(cq + 1) * SQC], in_=qps)
        return qtp

    def emit_ktp_chunk(ktp, p, ck):
        kps = psum.tile([P, SQC], F32, name="kps", tag="proj")
        for ci in range(NCI):
            nc.tensor.matmul(
                kps, lhsT=wkt_sb[ci][:, p * P:(p + 1) * P],
                rhs=xt_sb[ci][:, ck * SQC:(ck + 1) * SQC],
                start=(ci == 0), stop=(ci == NCI - 1))
        nc.vector.tensor_copy(out=ktp[:, ck * SQC:(ck + 1) * SQC], in_=kps)

    # Ot: normalized attention output, head-concat layout [c_in, sq]
    ot_sb = [const.tile([P, SQ], BF16, name=f"ot{i}", tag=f"ot{i}")
             for i in range(NCI)]

    def make_norm_tail(h, oraw, r):
        """Broadcast-matmul + normalize for head h. Deferred into the next
        head's loop so the PE-stream bcast matmul never waits on the DVE
        recip (PE is in-order; an early bcast would bubble the pipeline)."""
        def tail():
            rbp = psum.tile([D, SQ], F32, name="rbp", tag="st")
            for cq in range(NSQC):
                sl = slice(cq * SQC, (cq + 1) * SQC)
                nc.tensor.matmul(rbp[:, sl], lhsT=ones_sb[D:D + 1, :],
                                 rhs=r[D:D + 1, sl], start=True, stop=True)
            rb = work.tile([D, SQ], F32, name="rb", tag="rb", bufs=2)
            nc.vector.tensor_copy(out=rb, in_=rbp)
            if h % 2 == 0:
                nc.vector.tensor_mul(out=ot_sb[h // 2][0:D, :],
                                     in0=oraw[0:D, :], in1=rb)
            else:
                # DVE lanes are partition-locked; move to the upper half by DMA
                otmp = work.tile([D, SQ], BF16, name="otmp", tag="otmp",
                                 bufs=2)
                nc.vector.tensor_mul(out=otmp, in0=oraw[0:D, :], in1=rb)
                nc.gpsimd.dma_start(ot_sb[h // 2][D:2 * D, :], otmp)
        return tail

    outacc = const.tile([P, S], F32, name="outacc", tag="outacc")

    def make_oproj_tail(pair):
        """Accumulate pair `pair`'s output-projection contribution into
        outacc (SBUF). Deferred so only the final pair's slice is in the
        kernel tail."""
        def tail():
            for sqt in range(SQ // P):
                ops = psum.tile([P, C], F32, name="ops", tag="proj")
                nc.tensor.matmul(ops,
                                 lhsT=ot_sb[pair][:, sqt * P:(sqt + 1) * P],
                                 rhs=wot_sb[pair], start=True, stop=True)
                osl = outacc[:, sqt * C:(sqt + 1) * C]
                if pair == 0:
                    nc.vector.tensor_add(osl, ops, bob_sb)
                else:
                    nc.vector.tensor_add(osl, osl, ops)
            if pair == NCI - 1:
                for sqt in range(SQ // P):
                    nc.gpsimd.dma_start(
                        out_d[sqt * P:(sqt + 1) * P, :],
                        outacc[:, sqt * C:(sqt + 1) * C])
        return tail

    ktp = qtp = None
    pending_norm = None
    pending_oproj = None
    next_pair = None          # (qtp, ktp, n_chunks_pre_emitted) for pair p+1
    pre_chunks = 0
    for h in range(H):
        p, half = h // 2, h % 2
        lo, hi = half * D, half * D + D          # head's rows in pair tiles
        olo, ohi = D - half * D, 2 * D - half * D  # opposite half rows

        if half == 0:
            if next_pair is not None:
                qtp, ktp, pre_chunks = next_pair
                next_pair = None
            else:
                qtp = emit_qtp(p)
                ktp = work.tile([P, S], BF16, name="ktp", tag="ktp")
                pre_chunks = 0
        # per-head swap copies: same rows duplicated into the other
        # partition half so both sq-chunks can use disjoint PE row groups
        if row_pack:
            dma_eng = nc.gpsimd
            qts = work.tile([P, SQ], BF16, name="qts", tag="qts")
            dma_eng.dma_start(qts[olo:ohi, :], qtp[lo:hi, :])
            kts = work.tile([P, S], BF16, name="kts", tag="kts")

        def emit_k_chunk(ck):
            if half == 0 and ck >= pre_chunks:
                emit_ktp_chunk(ktp, p, ck)
            if row_pack:
                dma_eng.dma_start(
                    kts[olo:ohi, ck * SQC:(ck + 1) * SQC],
                    ktp[lo:hi, ck * SQC:(ck + 1) * SQC])

        emit_k_chunk(0)
        oacc = psum.tile([E, SQ], F32, name="oacc", tag="oacc", bufs=1)
        for t_i in range(NSK):
            # prefetch the next K chunk one window early so the QK matmuls
            # never wait on the projection->evict->swap-DMA chain
            if t_i % 4 == 1 and t_i // 4 + 1 < S // SQC:
                emit_k_chunk(t_i // 4 + 1)
            if vp_sb[t_i] is None:
                emit_vproj(t_i)
            if t_i == 8 and pending_norm is not None:
                h_prev, tail = pending_norm
                tail()
                pending_norm = None
                if h_prev % 2 == 1:
                    pending_oproj = make_oproj_tail(h_prev // 2)
            if t_i == 16 and pending_oproj is not None:
                pending_oproj()
                pending_oproj = None
            # prefetch the next pair's Q/K projections late in the second
            # head of the current pair, so the pair boundary never stalls
            # ScalarE on the projection chain
            if t_i == 24 and half == 1 and h + 1 < H and next_pair is None:
                nq = emit_qtp(p + 1)
                nk = work.tile([P, S], BF16, name="ktp", tag="ktp")
                for ck0 in range(2):
                    emit_ktp_chunk(nk, p + 1, ck0)
                next_pair = (nq, nk, 2)

            st = psum.tile([P, SQ], F32, name="st", tag="st", bufs=2)
            ksl = slice(t_i * P, (t_i + 1) * P)
            if row_pack:
                nc.tensor.matmul(st[:, 0:SQC], lhsT=ktp[lo:hi, ksl],
                                 rhs=qtp[lo:hi, 0:SQC],
                                 start=True, stop=True,
                                 tile_position=(lo, 0))
                nc.tensor.matmul(st[:, SQC:SQ], lhsT=kts[olo:ohi, ksl],
                                 rhs=qts[olo:ohi, SQC:SQ],
                                 start=True, stop=True,
                                 tile_position=(olo, 0))
            else:
                for cq in range(NSQC):
                    nc.tensor.matmul(
                        st[:, cq * SQC:(cq + 1) * SQC],
                        lhsT=ktp[lo:hi, ksl],
                        rhs=qtp[lo:hi, cq * SQC:(cq + 1) * SQC],
                        start=True, stop=True)
            pt = work.tile([P, SQ], BF16, name="pt", tag="pt", bufs=3)
            if dve_exp and t_i % 4 == 3:
                nc.vector._custom_dve(EXP16, out=pt, in0=st,
                                      s0=_EXPC[0], s1=_EXPC[1], imm2=_EXPC[2])
            else:
                nc.scalar.activation(out=pt, in_=st,
                                     func=mybir.ActivationFunctionType.Exp)
            for cq in range(NSQC):
                nc.tensor.matmul(
                    oacc[:, cq * SQC:(cq + 1) * SQC],
                    lhsT=vp_sb[t_i][:, h * E:(h + 1) * E],
                    rhs=pt[:, cq * SQC:(cq + 1) * SQC],
                    start=(t_i == 0), stop=(t_i == NSK - 1))

        # evict oacc to SBUF immediately so the PSUM slot frees for the next
        # head; the bcast+normalize runs deferred, off the critical path
        oraw = work.tile([E, SQ], F32, name="oraw", tag="oraw", bufs=2)
        nc.vector.tensor_copy(out=oraw, in_=oacc)
        r = work.tile([E, SQ], mybir.dt.float16, name="r", tag="r", bufs=2)
        with nc.allow_low_precision("softmax denom recip; fp16 ~1e-4 rel"):
            nc.vector.reciprocal(r[D:E, :], oraw[D:E, :])
        pending_norm = (h, make_norm_tail(h, oraw, r))

    if pending_oproj is not None:      # pair 2, if heads ended before t==16
        pending_oproj()
    pending_norm[1]()                  # final head's normalization
    make_oproj_tail(NCI - 1)()         # final pair's projection + store


def make_in_maps(hidden_states, Wq, Wk, Wv, Wo, bo):
    bf16 = ml_dtypes.bfloat16
    scale = np.float32(D) ** -0.5

    wqt = np.ascontiguousarray(Wq.T.astype(np.float32) * scale).astype(bf16)
    wkt = np.ascontiguousarray(Wk.T).astype(bf16)
    wvt = np.ascontiguousarray(Wv.T).astype(bf16)
    wot = np.ascontiguousarray(Wo.T).astype(bf16)
    bob = np.broadcast_to(np.asarray(bo, np.float32), (P, C)).copy()

    xt = [np.ascontiguousarray(np.asarray(hidden_states[b]).T).astype(bf16)
          for b in range(B)]

    in_maps = []
    for c in range(NCORES):
        b, q0 = c // 4, (c % 4) * SQ
        in_maps.append({
            "xt": xt[b],
            "xtq": np.ascontiguousarray(xt[b][:, q0:q0 + SQ]),
            "wqt": wqt, "wkt": wkt, "wvt": wvt, "wot": wot, "bob": bob,
        })
    return in_maps


_NC_CACHE = {}


def _get_nc():
    if "nc" not in _NC_CACHE:
        _NC_CACHE["nc"] = build_nc()
    return _NC_CACHE["nc"]


def run(inputs, trace=False, **kwargs):
    """Run on hardware; returns (full_output [B,S,C] fp32, BassKernelResults)."""
    nc = _get_nc()
    in_maps = make_in_maps(**inputs)
    res = run_bass_kernel_spmd(nc, in_maps, list(range(NCORES)), trace=trace,
                               **kwargs)
    out = np.empty((B, S, C), np.float32)
    for c in range(NCORES):
        b, q0 = c // 4, (c % 4) * SQ
        out[b, q0:q0 + SQ, :] = res.results[c]["out"]
    return out, res


def kernel(**inputs):
    out, _ = run(inputs)
    return out



# revision 7
# speedup vs baseline: 4.8385x; 4.8385x over previous
"""Trainium2 Bass kernel for diffusers AttnProcessor self-attention.

Reference computation (fp32, B=2, S=4096, C=512, H=8, D=64):
    q = hs @ Wq.T ; k = hs @ Wk.T ; v = hs @ Wv.T          (per-head split)
    probs = softmax(q k^T / sqrt(D))                        [b,h,s,s]
    out = (probs @ v) @ Wo.T + bo                           [b,s,c]

Sharding: 8 cores = (batch b in 0..1) x (query-slice of 1024 rows in 0..3).
Each core receives ONLY its own 1024-row Xt slice plus a 1/8 slice of the
packed projection weights; device-side AllGathers rebuild the full Xt per
batch group ([[0,1,2,3],[4,5,6,7]]) and the full weight pack ([[0..7]]), so
the host->device wire carries each input byte exactly once (~10MB total
instead of ~78MB replicated). Output returns as fp16 (~8MB).

Device dataflow per core (all matmuls bf16 in / fp32 PSUM accum):
  wg = AllGather(wsl)  [4C, C]   (wqt|wkt|wvt|wot pack, q-scale prefolded)
  xg = AllGather(xtq)  [4C, SQ]  (blocked Xt: block qb = Xt[:, qb*SQ:(qb+1)*SQ])
  Qt = wqt^T @ xtq  per head-pair                  [128, 1024]
  Kt = wkt^T @ xg   per head-pair                  [128, 4096]
  (a per-head copy of Qt/Kt rows is DMA'd to the opposite partition half so
   the two sq-chunks of the QK^T matmul run in disjoint PE row groups)
  V' = [xg @ wvt | 1] per head                     [S, 65] per head
  per head h, per key tile t (128 keys):
    St[t] = Kt_h[:,t]^T Qt_h        [128 sk, 1024 sq]  (2 row-packed matmuls)
    Pt    = exp(St)                 (ScalarE, bf16 out)
    O'_h += V'[t]^T Pt              [65, 1024]  (row 64 = softmax denominator)
  O_h = O'_h[0:64] * (1/O'_h[64])   -> Ot (head-concat layout)
  out = Ot^T @ wot + bob            -> DMA out  [1024, 512] fp16

Dispatch: a module-cached jax.jit(shard_map) closure over the bass_exec
custom call (re-creating it per call, as run_bass_kernel_spmd does, pays a
full retrace + lowering every time). Donated output buffers are generated
on-device by a tiny jitted zeros fn so they never cross the wire.
"""

import numpy as np
import ml_dtypes
from contextlib import ExitStack

import concourse.bass as bass
import concourse.bacc as bacc
import concourse.mybir as mybir
import concourse.tile as tile

BF16 = mybir.dt.bfloat16
F16 = mybir.dt.float16
F32 = mybir.dt.float32

B, S, C, H, D = 2, 4096, 512, 8, 64
NCORES = 8
SQ = 1024          # query rows per core
P = 128            # partitions
NSK = S // P       # 32 key tiles
NCI = C // P       # 4 contraction tiles for projections
SQC = 512          # matmul moving free dim
NSQC = SQ // SQC   # 2
E = D + 1          # V' columns per head (64 v cols + ones col)
WSL = 4 * C // NCORES  # weight-pack rows shipped per core (256)

ROW_PACK = True    # run the two K=64 QK^T matmuls in disjoint PE row groups


def build_nc(row_pack=ROW_PACK, reps=1):
    nc = bacc.Bacc("TRN2", target_bir_lowering=False, debug=False,
                   num_devices=NCORES)

    xtq_d = nc.dram_tensor("xtq", [C, SQ], BF16, kind="ExternalInput").ap()
    wsl_d = nc.dram_tensor("wsl", [WSL, C], BF16, kind="ExternalInput").ap()
    bo1_d = nc.dram_tensor("bo1", [1, C], F32, kind="ExternalInput").ap()
    out_d = nc.dram_tensor("out", [SQ, C], F16, kind="ExternalOutput").ap()
    wg_d = nc.dram_tensor("wg", [4 * C, C], BF16, kind="Internal",
                          addr_space="Shared").ap()
    # 4-core groups don't support Shared collective outputs (needs >4);
    # Local costs an extra runtime bounce but keeps the 2x4 gather.
    xg_d = nc.dram_tensor("xg", [4 * C, SQ], BF16, kind="Internal").ap()
    # collectives cannot read I/O tensors; bounce the inputs through
    # internal DRAM first
    wb_d = nc.dram_tensor("wb", [WSL, C], BF16, kind="Internal").ap()
    xb_d = nc.dram_tensor("xb", [C, SQ], BF16, kind="Internal").ap()

    with ExitStack() as ctx:
        tc = ctx.enter_context(tile.TileContext(nc))
        const = ctx.enter_context(tc.tile_pool(name="const", bufs=1))
        work = ctx.enter_context(tc.tile_pool(name="work", bufs=2))
        psum = ctx.enter_context(tc.tile_pool(name="psum", bufs=2, space="PSUM"))

        # Device-side gathers: weights first (first QK tile needs wqt+wkt),
        # then the batch group's Xt blocks. Collectives run on gpsimd in
        # issue order.
        nc.gpsimd.dma_start(wb_d, wsl_d)
        nc.gpsimd.dma_start(xb_d, xtq_d)
        nc.gpsimd.collective_compute(
            "AllGather", mybir.AluOpType.bypass,
            replica_groups=[list(range(NCORES))],
            ins=[wb_d], outs=[wg_d])
        nc.gpsimd.collective_compute(
            "AllGather", mybir.AluOpType.bypass,
            replica_groups=[[0, 1, 2, 3], [4, 5, 6, 7]],
            ins=[xb_d], outs=[xg_d])

        def load_tiles(dram_ap, base, row0, eng=None):
            tiles = []
            for ci in range(NCI):
                t = const.tile([P, C], BF16, name=f"{base}{ci}",
                               tag=f"{base}{ci}")
                r = row0 + ci * P
                (eng or nc.sync).dma_start(t, dram_ap[r:r + P, :])
                tiles.append(t)
            return tiles

        # Input loads split between the SP queue and the (startup-idle) ACT
        # queue, ordered by first use. Dependent SBUF<->SBUF moves go on the
        # gpsimd queue so they can't FIFO-block behind these.
        xtq_sb = []
        for ci in range(NCI):
            t = const.tile([P, SQ], BF16, name=f"xtqs{ci}", tag=f"xtqs{ci}")
            nc.scalar.dma_start(t, xtq_d[ci * P:(ci + 1) * P, :])
            xtq_sb.append(t)
        wqt_sb = load_tiles(wg_d, "wqts", 0 * C, eng=nc.scalar)
        wkt_sb = load_tiles(wg_d, "wkts", 1 * C)
        # xt_sb[ci] [P, S]: column chunk ck (512 wide) lives in gathered
        # block qb=ck//2 at local columns (ck%2)*SQC.
        xt_sb = [const.tile([P, S], BF16, name=f"xts{ci}", tag=f"xts{ci}")
                 for ci in range(NCI)]

        def load_xt_chunk(ci, ck):
            qb, loc = ck // 2, (ck % 2) * SQC
            nc.sync.dma_start(
                xt_sb[ci][:, ck * SQC:(ck + 1) * SQC],
                xg_d[qb * C + ci * P:qb * C + (ci + 1) * P, loc:loc + SQC])

        for ci in range(NCI):
            load_xt_chunk(ci, 0)
        wvt_sb = load_tiles(wg_d, "wvts", 2 * C)
        for ck in range(1, S // SQC):
            for ci in range(NCI):
                load_xt_chunk(ci, ck)
        wot_sb = load_tiles(wg_d, "wots", 3 * C)

        # bob = broadcast of bo to all 128 partitions via a K=1 ones-matmul
        bo1_sb = const.tile([1, C], F32, name="bo1s", tag="bo1s")
        nc.sync.dma_start(bo1_sb, bo1_d)
        one1_sb = const.tile([1, P], F32, name="one1", tag="one1")
        nc.vector.memset(one1_sb, 1.0)
        bob_ps = psum.tile([P, C], F32, name="bobp", tag="proj")
        nc.tensor.matmul(bob_ps, lhsT=one1_sb, rhs=bo1_sb,
                         start=True, stop=True)
        bob_sb = const.tile([P, C], F32, name="bobs", tag="bobs")
        nc.vector.tensor_copy(out=bob_sb, in_=bob_ps)

        ones_sb = const.tile([P, D], mybir.dt.float16, name="ones_sb",
                             tag="ones_sb")
        nc.vector.memset(ones_sb, 1.0)

        for rep in range(reps):
            emit_body(nc, tc, const, work, psum, row_pack,
                      xt_sb, xtq_sb, wqt_sb, wkt_sb, wvt_sb, wot_sb,
                      bob_sb, ones_sb, out_d)

    nc.compile()
    return nc


def emit_body(nc, tc, const, work, psum, row_pack,
              xt_sb, xtq_sb, wqt_sb, wkt_sb, wvt_sb, wot_sb,
              bob_sb, ones_sb, out_d):
    vp_sb = [None] * NSK

    def emit_vproj(t_i):
        vps = psum.tile([P, C], F32, name="vps", tag="proj")
        for ci in range(NCI):
            nc.tensor.matmul(vps, lhsT=xt_sb[ci][:, t_i * P:(t_i + 1) * P],
                             rhs=wvt_sb[ci],
                             start=(ci == 0), stop=(ci == NCI - 1))
        vp = const.tile([P, H * E], BF16, name=f"vp{t_i}", tag=f"vp{t_i}")
        vp3 = vp.rearrange("p (h e) -> p h e", e=E)
        nc.vector.tensor_copy(out=vp3[:, :, 0:D],
                              in_=vps.rearrange("p (h d) -> p h d", d=D))
        nc.vector.memset(vp3[:, :, D:E], 1.0)
        vp_sb[t_i] = vp

    def emit_qtp(p):
        qtp = work.tile([P, SQ], BF16, name="qtp", tag="qtp")
        for cq in range(NSQC):
            qps = psum.tile([P, SQC], F32, name="qps", tag="proj")
            for ci in range(NCI):
                nc.tensor.matmul(
                    qps, lhsT=wqt_sb[ci][:, p * P:(p + 1) * P],
                    rhs=xtq_sb[ci][:, cq * SQC:(cq + 1) * SQC],
                    start=(ci == 0), stop=(ci == NCI - 1))
            nc.vector.tensor_copy(out=qtp[:, cq * SQC:(cq + 1) * SQC], in_=qps)
        return qtp

    def emit_ktp_chunk(ktp, p, ck):
        kps = psum.tile([P, SQC], F32, name="kps", tag="proj")
        for ci in range(NCI):
            nc.tensor.matmul(
                kps, lhsT=wkt_sb[ci][:, p * P:(p + 1) * P],
                rhs=xt_sb[ci][:, ck * SQC:(ck + 1) * SQC],
                start=(ci == 0), stop=(ci == NCI - 1))
        nc.vector.tensor_copy(out=ktp[:, ck * SQC:(ck + 1) * SQC], in_=kps)

    # Ot: normalized attention output, head-concat layout [c_in, sq]
    ot_sb = [const.tile([P, SQ], BF16, name=f"ot{i}", tag=f"ot{i}")
             for i in range(NCI)]

    def make_norm_tail(h, oraw, r):
        """Broadcast-matmul + normalize for head h. Deferred into the next
        head's loop so the PE-stream bcast matmul never waits on the DVE
        recip (PE is in-order; an early bcast would bubble the pipeline)."""
        def tail():
            rbp = psum.tile([D, SQ], F32, name="rbp", tag="st")
            for cq in range(NSQC):
                sl = slice(cq * SQC, (cq + 1) * SQC)
                nc.tensor.matmul(rbp[:, sl], lhsT=ones_sb[D:D + 1, :],
                                 rhs=r[D:D + 1, sl], start=True, stop=True)
            rb = work.tile([D, SQ], F32, name="rb", tag="rb", bufs=2)
            nc.vector.tensor_copy(out=rb, in_=rbp)
            if h % 2 == 0:
                nc.vector.tensor_mul(out=ot_sb[h // 2][0:D, :],
                                     in0=oraw[0:D, :], in1=rb)
            else:
                # DVE lanes are partition-locked; move to the upper half by DMA
                otmp = work.tile([D, SQ], BF16, name="otmp", tag="otmp",
                                 bufs=2)
                nc.vector.tensor_mul(out=otmp, in0=oraw[0:D, :], in1=rb)
                nc.gpsimd.dma_start(ot_sb[h // 2][D:2 * D, :], otmp)
        return tail

    outacc = const.tile([P, S], F16, name="outacc", tag="outacc")

    def make_oproj_tail(pair):
        """Accumulate pair `pair`'s output-projection contribution into
        outacc (SBUF). Deferred so only the final pair's slice is in the
        kernel tail."""
        def tail():
            for sqt in range(SQ // P):
                ops = psum.tile([P, C], F32, name="ops", tag="proj")
                nc.tensor.matmul(ops,
                                 lhsT=ot_sb[pair][:, sqt * P:(sqt + 1) * P],
                                 rhs=wot_sb[pair], start=True, stop=True)
                osl = outacc[:, sqt * C:(sqt + 1) * C]
                if pair == 0:
                    nc.vector.tensor_add(osl, ops, bob_sb)
                else:
                    nc.vector.tensor_add(osl, osl, ops)
            if pair == NCI - 1:
                for sqt in range(SQ // P):
                    nc.gpsimd.dma_start(
                        out_d[sqt * P:(sqt + 1) * P, :],
                        outacc[:, sqt * C:(sqt + 1) * C])
        return tail

    ktp = qtp = None
    pending_norm = None
    pending_oproj = None
    next_pair = None          # (qtp, ktp, n_chunks_pre_emitted) for pair p+1
    pre_chunks = 0
    for h in range(H):
        p, half = h // 2, h % 2
        lo, hi = half * D, half * D + D          # head's rows in pair tiles
        olo, ohi = D - half * D, 2 * D - half * D  # opposite half rows

        if half == 0:
            if next_pair is not None:
                qtp, ktp, pre_chunks = next_pair
                next_pair = None
            else:
                qtp = emit_qtp(p)
                ktp = work.tile([P, S], BF16, name="ktp", tag="ktp")
                pre_chunks = 0
        # per-head swap copies: same rows duplicated into the other
        # partition half so both sq-chunks can use disjoint PE row groups
        if row_pack:
            dma_eng = nc.gpsimd
            qts = work.tile([P, SQ], BF16, name="qts", tag="qts")
            dma_eng.dma_start(qts[olo:ohi, :], qtp[lo:hi, :])
            kts = work.tile([P, S], BF16, name="kts", tag="kts")

        def emit_k_chunk(ck):
            if half == 0 and ck >= pre_chunks:
                emit_ktp_chunk(ktp, p, ck)
            if row_pack:
                dma_eng.dma_start(
                    kts[olo:ohi, ck * SQC:(ck + 1) * SQC],
                    ktp[lo:hi, ck * SQC:(ck + 1) * SQC])

        emit_k_chunk(0)
        oacc = psum.tile([E, SQ], F32, name="oacc", tag="oacc", bufs=1)
        for t_i in range(NSK):
            # prefetch the next K chunk one window early so the QK matmuls
            # never wait on the projection->evict->swap-DMA chain
            if t_i % 4 == 1 and t_i // 4 + 1 < S // SQC:
                emit_k_chunk(t_i // 4 + 1)
            if vp_sb[t_i] is None:
                emit_vproj(t_i)
            if t_i == 8 and pending_norm is not None:
                h_prev, tail = pending_norm
                tail()
                pending_norm = None
                if h_prev % 2 == 1:
                    pending_oproj = make_oproj_tail(h_prev // 2)
            if t_i == 16 and pending_oproj is not None:
                pending_oproj()
                pending_oproj = None
            # prefetch the next pair's Q/K projections late in the second
            # head of the current pair, so the pair boundary never stalls
            # ScalarE on the projection chain
            if t_i == 24 and half == 1 and h + 1 < H and next_pair is None:
                nq = emit_qtp(p + 1)
                nk = work.tile([P, S], BF16, name="ktp", tag="ktp")
                for ck0 in range(2):
                    emit_ktp_chunk(nk, p + 1, ck0)
                next_pair = (nq, nk, 2)

            st = psum.tile([P, SQ], F32, name="st", tag="st", bufs=2)
            ksl = slice(t_i * P, (t_i + 1) * P)
            if row_pack:
                nc.tensor.matmul(st[:, 0:SQC], lhsT=ktp[lo:hi, ksl],
                                 rhs=qtp[lo:hi, 0:SQC],
                                 start=True, stop=True,
                                 tile_position=(lo, 0))
                nc.tensor.matmul(st[:, SQC:SQ], lhsT=kts[olo:ohi, ksl],
                                 rhs=qts[olo:ohi, SQC:SQ],
                                 start=True, stop=True,
                                 tile_position=(olo, 0))
            else:
                for cq in range(NSQC):
                    nc.tensor.matmul(
                        st[:, cq * SQC:(cq + 1) * SQC],
                        lhsT=ktp[lo:hi, ksl],
                        rhs=qtp[lo:hi, cq * SQC:(cq + 1) * SQC],
                        start=True, stop=True)
            pt = work.tile([P, SQ], BF16, name="pt", tag="pt", bufs=3)
            nc.scalar.activation(out=pt, in_=st,
                                 func=mybir.ActivationFunctionType.Exp)
            for cq in range(NSQC):
                nc.tensor.matmul(
                    oacc[:, cq * SQC:(cq + 1) * SQC],
                    lhsT=vp_sb[t_i][:, h * E:(h + 1) * E],
                    rhs=pt[:, cq * SQC:(cq + 1) * SQC],
                    start=(t_i == 0), stop=(t_i == NSK - 1))

        # evict oacc to SBUF immediately so the PSUM slot frees for the next
        # head; the bcast+normalize runs deferred, off the critical path
        oraw = work.tile([E, SQ], F32, name="oraw", tag="oraw", bufs=2)
        nc.vector.tensor_copy(out=oraw, in_=oacc)
        r = work.tile([E, SQ], mybir.dt.float16, name="r", tag="r", bufs=2)
        with nc.allow_low_precision("softmax denom recip; fp16 ~1e-4 rel"):
            nc.vector.reciprocal(r[D:E, :], oraw[D:E, :])
        pending_norm = (h, make_norm_tail(h, oraw, r))

    if pending_oproj is not None:      # pair 2, if heads ended before t==16
        pending_oproj()
    pending_norm[1]()                  # final head's normalization
    make_oproj_tail(NCI - 1)()         # final pair's projection + store


def make_cat_inputs(hidden_states, Wq, Wk, Wv, Wo, bo):
    """Build the already-concatenated global input arrays (axis 0 = core)."""
    bf16 = ml_dtypes.bfloat16
    scale = np.float32(D) ** -0.5

    cat_x = np.empty((NCORES * C, SQ), bf16)
    for b in range(B):
        xt = np.asarray(hidden_states[b]).T  # strided fp32 view [C, S]
        for qi in range(S // SQ):
            r0 = (b * (S // SQ) + qi) * C
            cat_x[r0:r0 + C] = xt[:, qi * SQ:(qi + 1) * SQ]

    cat_w = np.empty((4 * C, C), bf16)
    cat_w[0 * C:1 * C] = np.asarray(Wq).T * scale
    cat_w[1 * C:2 * C] = np.asarray(Wk).T
    cat_w[2 * C:3 * C] = np.asarray(Wv).T
    cat_w[3 * C:4 * C] = np.asarray(Wo).T

    cat_bo = np.broadcast_to(
        np.asarray(bo, np.float32), (NCORES, C)).copy()

    return {"xtq": cat_x, "wsl": cat_w, "bo1": cat_bo}


def _build_dispatch():
    """Compile the Bass module and build the cached jit dispatch closure."""
    import jax
    import jax.numpy as jnp
    from jax.sharding import Mesh, PartitionSpec, NamedSharding
    from jax.experimental.shard_map import shard_map
    from concourse.bass2jax import (
        _bass_exec_p, partition_id_tensor, install_neuronx_cc_hook)

    nc = build_nc()
    install_neuronx_cc_hook()
    assert nc.dbg_addr is None, "build with debug=False"

    partition_name = (nc.partition_id_tensor.name
                      if nc.partition_id_tensor else None)
    in_names, out_names, out_avals = [], [], []
    for alloc in nc.m.functions[0].allocations:
        if not isinstance(alloc, mybir.MemoryLocationSet):
            continue
        name = alloc.memorylocations[0].name
        if alloc.kind == "ExternalInput":
            if name != partition_name:
                in_names.append(name)
        elif alloc.kind == "ExternalOutput":
            shape = tuple(alloc.tensor_shape)
            dtype = mybir.dt.np(alloc.dtype)
            out_names.append(name)
            out_avals.append(jax.core.ShapedArray(shape, dtype))
    n_params = len(in_names)
    n_outs = len(out_avals)
    all_names = in_names + out_names + (
        [partition_name] if partition_name else [])
    donate = tuple(range(n_params, n_params + n_outs))

    def _body(*args):
        operands = list(args)
        if partition_name is not None:
            operands.append(partition_id_tensor())
        outs = _bass_exec_p.bind(
            *operands,
            out_avals=tuple(out_avals),
            in_names=tuple(all_names),
            out_names=tuple(out_names),
            lowering_input_output_aliases=(),
            sim_require_finite=True,
            sim_require_nnan=True,
            nc=nc,
        )
        return tuple(outs)

    devices = jax.devices()[:NCORES]
    mesh = Mesh(np.asarray(devices), ("core",))
    in_specs = (PartitionSpec("core"),) * (n_params + n_outs)
    out_specs = (PartitionSpec("core"),) * n_outs
    sharded = jax.jit(
        shard_map(_body, mesh=mesh, in_specs=in_specs, out_specs=out_specs,
                  check_rep=False),
        donate_argnums=donate, keep_unused=True)

    # Donated output buffers, generated on-device (never cross the wire).
    zero_shardings = tuple(
        NamedSharding(mesh, PartitionSpec("core")) for _ in range(n_outs))
    zeros_fn = jax.jit(
        lambda: tuple(
            jnp.zeros((NCORES * a.shape[0], *a.shape[1:]), a.dtype)
            for a in out_avals),
        out_shardings=zero_shardings)

    return {
        "nc": nc,
        "sharded": sharded,
        "zeros_fn": zeros_fn,
        "in_names": in_names,
        "out_names": out_names,
        "out_avals": out_avals,
    }


_CACHE = {}


def _get_dispatch():
    if "d" not in _CACHE:
        _CACHE["d"] = _build_dispatch()
    return _CACHE["d"]


def run(inputs, trace=False, **kwargs):
    """Run on hardware; returns (full_output [B,S,C] fp32, aux)."""
    d = _get_dispatch()
    cat = make_cat_inputs(**inputs)
    zeros = d["zeros_fn"]()
    out_arrs = d["sharded"](*[cat[n] for n in d["in_names"]], *zeros)
    out_g = np.asarray(out_arrs[d["out_names"].index("out")])
    out = out_g.reshape(B, 4, SQ, C).astype(np.float32).reshape(B, S, C)
    return out, None


def kernel(**inputs):
    out, _ = run(inputs)
    return out


# revision 11
# speedup vs baseline: 5.1778x; 1.0701x over previous
"""Trainium2 Bass kernel for diffusers AttnProcessor self-attention.

Reference computation (fp32, B=2, S=4096, C=512, H=8, D=64):
    q = hs @ Wq.T ; k = hs @ Wk.T ; v = hs @ Wv.T          (per-head split)
    probs = softmax(q k^T / sqrt(D))                        [b,h,s,s]
    out = (probs @ v) @ Wo.T + bo                           [b,s,c]

Sharding: 8 cores = (batch b in 0..1) x (query-slice of 1024 rows in 0..3).
The two batch groups are compiled as INDEPENDENT 4-core launches (meshes
over devices 0-3 and 4-7) so the axon tunnel — which is full-duplex but
only ~40MB/s each way and dominates wall-clock — can overlap batch 0's
output download with batch 1's input upload.

Each core receives ONLY its own 1024-row Xt slice plus a 1/4 slice of the
packed projection weights (+ a bo row); device-side 4-way AllGathers
rebuild the full blocked Xt and weight pack, so the host->device wire
carries each input byte exactly once per half. Output returns as fp16.

Device dataflow per core (all matmuls bf16 in / fp32 PSUM accum):
  wg = AllGather(wsl)  [4*513, C]  (wqt|wkt|wvt|wot blocks, q-scale folded,
                                    one bo row per block)
  xg = AllGather(xtq)  [4C, SQ]    (blocked Xt: block qb = Xt[:, qb*SQ:..])
  Qt = wqt^T @ xtq  per head-pair                  [128, 1024]
  Kt = wkt^T @ xg   per head-pair                  [128, 4096]
  (a per-head copy of Qt/Kt rows is DMA'd to the opposite partition half so
   the two sq-chunks of the QK^T matmul run in disjoint PE row groups)
  V' = [xg @ wvt | 1] per head                     [S, 65] per head
  per head h, per key tile t (128 keys):
    St[t] = Kt_h[:,t]^T Qt_h        [128 sk, 1024 sq]  (2 row-packed matmuls)
    Pt    = exp(St)                 (ScalarE, bf16 out)
    O'_h += V'[t]^T Pt              [65, 1024]  (row 64 = softmax denominator)
  O_h = O'_h[0:64] * (1/O'_h[64])   -> Ot (head-concat layout)
  out = Ot^T @ wot + bob            -> DMA out  [1024, 512] fp16

Dispatch: module-cached jax.jit(shard_map) closures (one per half) over the
bass_exec custom call. Output operands are device-resident dummies reused
across calls (the kernel writes every element of `out`, so their contents
never matter and they never cross the wire).
"""

import numpy as np
import ml_dtypes
from contextlib import ExitStack
from concurrent.futures import ThreadPoolExecutor

import concourse.bass as bass
import concourse.bacc as bacc
import concourse.mybir as mybir
import concourse.tile as tile

BF16 = mybir.dt.bfloat16
F16 = mybir.dt.float16
F32 = mybir.dt.float32

B, S, C, H, D = 2, 4096, 512, 8, 64
NCORES = 8
NHALF = 4          # cores per batch group / per launch
SQ = 1024          # query rows per core
P = 128            # partitions
NSK = S // P       # 32 key tiles
NCI = C // P       # 4 contraction tiles for projections
SQC = 512          # matmul moving free dim
NSQC = SQ // SQC   # 2
E = D + 1          # V' columns per head (64 v cols + ones col)
WROW = C + 1       # weight-pack rows shipped per core (one weight + bo row)

ROW_PACK = True    # run the two K=64 QK^T matmuls in disjoint PE row groups


def build_nc(row_pack=ROW_PACK, reps=1):
    # num_devices=8 with BOTH groups listed: NRT resolves a device's
    # replica group by GLOBAL device id, so one NEFF serves both 4-device
    # launches (devices 0-3 -> group 0, devices 4-7 -> group 1).
    nc = bacc.Bacc("TRN2", target_bir_lowering=False, debug=False,
                   num_devices=NCORES)

    xtq_d = nc.dram_tensor("xtq", [C, SQ], BF16, kind="ExternalInput").ap()
    wsl_d = nc.dram_tensor("wsl", [WROW, C], BF16, kind="ExternalInput").ap()
    out_d = nc.dram_tensor("out", [SQ, C], F16, kind="ExternalOutput").ap()
    # 4-core groups don't support Shared collective outputs (needs >4)
    wg_d = nc.dram_tensor("wg", [NHALF * WROW, C], BF16, kind="Internal").ap()
    xg_d = nc.dram_tensor("xg", [4 * C, SQ], BF16, kind="Internal").ap()
    # collectives cannot read I/O tensors; bounce the inputs through
    # internal DRAM first
    wb_d = nc.dram_tensor("wb", [WROW, C], BF16, kind="Internal").ap()
    xb_d = nc.dram_tensor("xb", [C, SQ], BF16, kind="Internal").ap()

    with ExitStack() as ctx:
        tc = ctx.enter_context(tile.TileContext(nc))
        const = ctx.enter_context(tc.tile_pool(name="const", bufs=1))
        work = ctx.enter_context(tc.tile_pool(name="work", bufs=2))
        psum = ctx.enter_context(tc.tile_pool(name="psum", bufs=2, space="PSUM"))

        # Device-side gathers: weights first (first QK tile needs wqt+wkt),
        # then the batch group's Xt blocks. Collectives run on gpsimd in
        # issue order.
        groups = [[0, 1, 2, 3], [4, 5, 6, 7]]
        nc.gpsimd.dma_start(wb_d, wsl_d)
        nc.gpsimd.dma_start(xb_d, xtq_d)
        nc.gpsimd.collective_compute(
            "AllGather", mybir.AluOpType.bypass,
            replica_groups=groups, ins=[wb_d], outs=[wg_d])
        nc.gpsimd.collective_compute(
            "AllGather", mybir.AluOpType.bypass,
            replica_groups=groups, ins=[xb_d], outs=[xg_d])

        def load_tiles(base_name, m, eng=None):
            # weight m's [C, C] block lives at gathered rows m*WROW..+C
            tiles = []
            for ci in range(NCI):
                t = const.tile([P, C], BF16, name=f"{base_name}{ci}",
                               tag=f"{base_name}{ci}")
                r = m * WROW + ci * P
                (eng or nc.sync).dma_start(t, wg_d[r:r + P, :])
                tiles.append(t)
            return tiles

        # Input loads split between the SP queue and the (startup-idle) ACT
        # queue, ordered by first use. Dependent SBUF<->SBUF moves go on the
        # gpsimd queue so they can't FIFO-block behind these.
        xtq_sb = []
        for ci in range(NCI):
            t = const.tile([P, SQ], BF16, name=f"xtqs{ci}", tag=f"xtqs{ci}")
            nc.scalar.dma_start(t, xtq_d[ci * P:(ci + 1) * P, :])
            xtq_sb.append(t)
        wqt_sb = load_tiles("wqts", 0, eng=nc.scalar)
        wkt_sb = load_tiles("wkts", 1)
        # xt_sb[ci] [P, S]: column chunk ck (512 wide) lives in gathered
        # block qb=ck//2 at local columns (ck%2)*SQC.
        xt_sb = [const.tile([P, S], BF16, name=f"xts{ci}", tag=f"xts{ci}")
                 for ci in range(NCI)]

        def load_xt_chunk(ci, ck):
            qb, loc = ck // 2, (ck % 2) * SQC
            nc.sync.dma_start(
                xt_sb[ci][:, ck * SQC:(ck + 1) * SQC],
                xg_d[qb * C + ci * P:qb * C + (ci + 1) * P, loc:loc + SQC])

        for ci in range(NCI):
            load_xt_chunk(ci, 0)
        wvt_sb = load_tiles("wvts", 2)
        for ck in range(1, S // SQC):
            for ci in range(NCI):
                load_xt_chunk(ci, ck)
        wot_sb = load_tiles("wots", 3)

        # bob = broadcast of bo (own input's last pack row, no gather
        # needed) to all 128 partitions via a K=1 ones-matmul
        bo1_sb = const.tile([1, C], BF16, name="bo1s", tag="bo1s")
        nc.sync.dma_start(bo1_sb, wsl_d[C:C + 1, :])
        one1_sb = const.tile([1, P], BF16, name="one1", tag="one1")
        nc.vector.memset(one1_sb, 1.0)
        bob_ps = psum.tile([P, C], F32, name="bobp", tag="proj")
        nc.tensor.matmul(bob_ps, lhsT=one1_sb, rhs=bo1_sb,
                         start=True, stop=True)
        bob_sb = const.tile([P, C], F32, name="bobs", tag="bobs")
        nc.vector.tensor_copy(out=bob_sb, in_=bob_ps)

        ones_sb = const.tile([P, D], mybir.dt.float16, name="ones_sb",
                             tag="ones_sb")
        nc.vector.memset(ones_sb, 1.0)

        for rep in range(reps):
            emit_body(nc, tc, const, work, psum, row_pack,
                      xt_sb, xtq_sb, wqt_sb, wkt_sb, wvt_sb, wot_sb,
                      bob_sb, ones_sb, out_d)

    nc.compile()
    return nc


def emit_body(nc, tc, const, work, psum, row_pack,
              xt_sb, xtq_sb, wqt_sb, wkt_sb, wvt_sb, wot_sb,
              bob_sb, ones_sb, out_d):
    vp_sb = [None] * NSK

    def emit_vproj(t_i):
        vps = psum.tile([P, C], F32, name="vps", tag="proj")
        for ci in range(NCI):
            nc.tensor.matmul(vps, lhsT=xt_sb[ci][:, t_i * P:(t_i + 1) * P],
                             rhs=wvt_sb[ci],
                             start=(ci == 0), stop=(ci == NCI - 1))
        vp = const.tile([P, H * E], BF16, name=f"vp{t_i}", tag=f"vp{t_i}")
        vp3 = vp.rearrange("p (h e) -> p h e", e=E)
        nc.vector.tensor_copy(out=vp3[:, :, 0:D],
                              in_=vps.rearrange("p (h d) -> p h d", d=D))
        nc.vector.memset(vp3[:, :, D:E], 1.0)
        vp_sb[t_i] = vp

    def emit_qtp(p):
        qtp = work.tile([P, SQ], BF16, name="qtp", tag="qtp")
        for cq in range(NSQC):
            qps = psum.tile([P, SQC], F32, name="qps", tag="proj")
            for ci in range(NCI):
                nc.tensor.matmul(
                    qps, lhsT=wqt_sb[ci][:, p * P:(p + 1) * P],
                    rhs=xtq_sb[ci][:, cq * SQC:(cq + 1) * SQC],
                    start=(ci == 0), stop=(ci == NCI - 1))
            nc.vector.tensor_copy(out=qtp[:, cq * SQC:(cq + 1) * SQC], in_=qps)
        return qtp

    def emit_ktp_chunk(ktp, p, ck):
        kps = psum.tile([P, SQC], F32, name="kps", tag="proj")
        for ci in range(NCI):
            nc.tensor.matmul(
                kps, lhsT=wkt_sb[ci][:, p * P:(p + 1) * P],
                rhs=xt_sb[ci][:, ck * SQC:(ck + 1) * SQC],
                start=(ci == 0), stop=(ci == NCI - 1))
        nc.vector.tensor_copy(out=ktp[:, ck * SQC:(ck + 1) * SQC], in_=kps)

    # Ot: normalized attention output, head-concat layout [c_in, sq]
    ot_sb = [const.tile([P, SQ], BF16, name=f"ot{i}", tag=f"ot{i}")
             for i in range(NCI)]

    def make_norm_tail(h, oraw, r):
        """Broadcast-matmul + normalize for head h. Deferred into the next
        head's loop so the PE-stream bcast matmul never waits on the DVE
        recip (PE is in-order; an early bcast would bubble the pipeline)."""
        def tail():
            rbp = psum.tile([D, SQ], F32, name="rbp", tag="st")
            for cq in range(NSQC):
                sl = slice(cq * SQC, (cq + 1) * SQC)
                nc.tensor.matmul(rbp[:, sl], lhsT=ones_sb[D:D + 1, :],
                                 rhs=r[D:D + 1, sl], start=True, stop=True)
            rb = work.tile([D, SQ], F32, name="rb", tag="rb", bufs=2)
            nc.vector.tensor_copy(out=rb, in_=rbp)
            if h % 2 == 0:
                nc.vector.tensor_mul(out=ot_sb[h // 2][0:D, :],
                                     in0=oraw[0:D, :], in1=rb)
            else:
                # DVE lanes are partition-locked; move to the upper half by DMA
                otmp = work.tile([D, SQ], BF16, name="otmp", tag="otmp",
                                 bufs=2)
                nc.vector.tensor_mul(out=otmp, in0=oraw[0:D, :], in1=rb)
                nc.gpsimd.dma_start(ot_sb[h // 2][D:2 * D, :], otmp)
        return tail

    outacc = const.tile([P, S], F16, name="outacc", tag="outacc")

    def make_oproj_tail(pair):
        """Accumulate pair `pair`'s output-projection contribution into
        outacc (SBUF). Deferred so only the final pair's slice is in the
        kernel tail."""
        def tail():
            for sqt in range(SQ // P):
                ops = psum.tile([P, C], F32, name="ops", tag="proj")
                nc.tensor.matmul(ops,
                                 lhsT=ot_sb[pair][:, sqt * P:(sqt + 1) * P],
                                 rhs=wot_sb[pair], start=True, stop=True)
                osl = outacc[:, sqt * C:(sqt + 1) * C]
                if pair == 0:
                    nc.vector.tensor_add(osl, ops, bob_sb)
                else:
                    nc.vector.tensor_add(osl, osl, ops)
            if pair == NCI - 1:
                for sqt in range(SQ // P):
                    nc.gpsimd.dma_start(
                        out_d[sqt * P:(sqt + 1) * P, :],
                        outacc[:, sqt * C:(sqt + 1) * C])
        return tail

    ktp = qtp = None
    pending_norm = None
    pending_oproj = None
    next_pair = None          # (qtp, ktp, n_chunks_pre_emitted) for pair p+1
    pre_chunks = 0
    for h in range(H):
        p, half = h // 2, h % 2
        lo, hi = half * D, half * D + D          # head's rows in pair tiles
        olo, ohi = D - half * D, 2 * D - half * D  # opposite half rows

        if half == 0:
            if next_pair is not None:
                qtp, ktp, pre_chunks = next_pair
                next_pair = None
            else:
                qtp = emit_qtp(p)
                ktp = work.tile([P, S], BF16, name="ktp", tag="ktp")
                pre_chunks = 0
        # per-head swap copies: same rows duplicated into the other
        # partition half so both sq-chunks can use disjoint PE row groups
        if row_pack:
            dma_eng = nc.gpsimd
            qts = work.tile([P, SQ], BF16, name="qts", tag="qts")
            dma_eng.dma_start(qts[olo:ohi, :], qtp[lo:hi, :])
            kts = work.tile([P, S], BF16, name="kts", tag="kts")

        def emit_k_chunk(ck):
            if half == 0 and ck >= pre_chunks:
                emit_ktp_chunk(ktp, p, ck)
            if row_pack:
                dma_eng.dma_start(
                    kts[olo:ohi, ck * SQC:(ck + 1) * SQC],
                    ktp[lo:hi, ck * SQC:(ck + 1) * SQC])

        emit_k_chunk(0)
        oacc = psum.tile([E, SQ], F32, name="oacc", tag="oacc", bufs=1)
        for t_i in range(NSK):
            # prefetch the next K chunk one window early so the QK matmuls
            # never wait on the projection->evict->swap-DMA chain
            if t_i % 4 == 1 and t_i // 4 + 1 < S // SQC:
                emit_k_chunk(t_i // 4 + 1)
            if vp_sb[t_i] is None:
                emit_vproj(t_i)
            if t_i == 8 and pending_norm is not None:
                h_prev, tail = pending_norm
                tail()
                pending_norm = None
                if h_prev % 2 == 1:
                    pending_oproj = make_oproj_tail(h_prev // 2)
            if t_i == 16 and pending_oproj is not None:
                pending_oproj()
                pending_oproj = None
            # prefetch the next pair's Q/K projections late in the second
            # head of the current pair, so the pair boundary never stalls
            # ScalarE on the projection chain
            if t_i == 24 and half == 1 and h + 1 < H and next_pair is None:
                nq = emit_qtp(p + 1)
                nk = work.tile([P, S], BF16, name="ktp", tag="ktp")
                for ck0 in range(2):
                    emit_ktp_chunk(nk, p + 1, ck0)
                next_pair = (nq, nk, 2)

            st = psum.tile([P, SQ], F32, name="st", tag="st", bufs=2)
            ksl = slice(t_i * P, (t_i + 1) * P)
            if row_pack:
                nc.tensor.matmul(st[:, 0:SQC], lhsT=ktp[lo:hi, ksl],
                                 rhs=qtp[lo:hi, 0:SQC],
                                 start=True, stop=True,
                                 tile_position=(lo, 0))
                nc.tensor.matmul(st[:, SQC:SQ], lhsT=kts[olo:ohi, ksl],
                                 rhs=qts[olo:ohi, SQC:SQ],
                                 start=True, stop=True,
                                 tile_position=(olo, 0))
            else:
                for cq in range(NSQC):
                    nc.tensor.matmul(
                        st[:, cq * SQC:(cq + 1) * SQC],
                        lhsT=ktp[lo:hi, ksl],
                        rhs=qtp[lo:hi, cq * SQC:(cq + 1) * SQC],
                        start=True, stop=True)
            pt = work.tile([P, SQ], BF16, name="pt", tag="pt", bufs=3)
            nc.scalar.activation(out=pt, in_=st,
                                 func=mybir.ActivationFunctionType.Exp)
            for cq in range(NSQC):
                nc.tensor.matmul(
                    oacc[:, cq * SQC:(cq + 1) * SQC],
                    lhsT=vp_sb[t_i][:, h * E:(h + 1) * E],
                    rhs=pt[:, cq * SQC:(cq + 1) * SQC],
                    start=(t_i == 0), stop=(t_i == NSK - 1))

        # evict oacc to SBUF immediately so the PSUM slot frees for the next
        # head; the bcast+normalize runs deferred, off the critical path
        oraw = work.tile([E, SQ], F32, name="oraw", tag="oraw", bufs=2)
        nc.vector.tensor_copy(out=oraw, in_=oacc)
        r = work.tile([E, SQ], mybir.dt.float16, name="r", tag="r", bufs=2)
        with nc.allow_low_precision("softmax denom recip; fp16 ~1e-4 rel"):
            nc.vector.reciprocal(r[D:E, :], oraw[D:E, :])
        pending_norm = (h, make_norm_tail(h, oraw, r))

    if pending_oproj is not None:      # pair 2, if heads ended before t==16
        pending_oproj()
    pending_norm[1]()                  # final head's normalization
    make_oproj_tail(NCI - 1)()         # final pair's projection + store


def make_w_cat(Wq, Wk, Wv, Wo, bo):
    """Packed weight array [4*WROW, C] bf16, shared by both halves.
    Block m = [weight m transposed (q-scale folded for m=0); bo row]."""
    bf16 = ml_dtypes.bfloat16
    scale = np.float32(D) ** -0.5
    cat_w = np.empty((NHALF * WROW, C), bf16)
    bob = np.asarray(bo, np.float32)
    for m, w in enumerate((Wq, Wk, Wv, Wo)):
        blk = cat_w[m * WROW:(m + 1) * WROW]
        wt = np.asarray(w).T
        blk[0:C] = wt * scale if m == 0 else wt
        blk[C] = bob
    return cat_w


def make_x_cat(hidden_states, b):
    """Blocked-Xt input for batch b: [4*C, SQ] bf16 (block qi = Xt slice)."""
    bf16 = ml_dtypes.bfloat16
    cat_x = np.empty((NHALF * C, SQ), bf16)
    xt = np.asarray(hidden_states[b]).T  # strided fp32 view [C, S]
    for qi in range(NHALF):
        cat_x[qi * C:(qi + 1) * C] = xt[:, qi * SQ:(qi + 1) * SQ]
    return cat_x


def _build_dispatch():
    """Compile the Bass module and build the cached jit dispatch closures."""
    import jax
    import jax.numpy as jnp
    from jax.sharding import Mesh, PartitionSpec, NamedSharding
    from jax.experimental.shard_map import shard_map
    from concourse.bass2jax import (
        _bass_exec_p, partition_id_tensor, install_neuronx_cc_hook)

    nc = build_nc()
    install_neuronx_cc_hook()
    assert nc.dbg_addr is None, "build with debug=False"

    partition_name = (nc.partition_id_tensor.name
                      if nc.partition_id_tensor else None)
    in_names, out_names, out_avals = [], [], []
    for alloc in nc.m.functions[0].allocations:
        if not isinstance(alloc, mybir.MemoryLocationSet):
            continue
        name = alloc.memorylocations[0].name
        if alloc.kind == "ExternalInput":
            if name != partition_name:
                in_names.append(name)
        elif alloc.kind == "ExternalOutput":
            shape = tuple(alloc.tensor_shape)
            dtype = mybir.dt.np(alloc.dtype)
            out_names.append(name)
            out_avals.append(jax.core.ShapedArray(shape, dtype))
    n_params = len(in_names)
    n_outs = len(out_avals)
    all_names = in_names + out_names + (
        [partition_name] if partition_name else [])

    def _body(*args):
        operands = list(args)
        if partition_name is not None:
            operands.append(partition_id_tensor())
        outs = _bass_exec_p.bind(
            *operands,
            out_avals=tuple(out_avals),
            in_names=tuple(all_names),
            out_names=tuple(out_names),
            lowering_input_output_aliases=(),
            sim_require_finite=True,
            sim_require_nnan=True,
            nc=nc,
        )
        return tuple(outs)

    devices = jax.devices()[:NCORES]
    halves = []
    for hi in range(2):
        mesh = Mesh(np.asarray(devices[hi * NHALF:(hi + 1) * NHALF]),
                    ("core",))
        in_specs = (PartitionSpec("core"),) * (n_params + n_outs)
        out_specs = (PartitionSpec("core"),) * n_outs
        # No donation: the kernel writes every element of `out`, so the
        # output operands are never read and can be device-resident dummies
        # reused across calls (zero wire traffic, zero per-call work).
        sharded = jax.jit(
            shard_map(_body, mesh=mesh, in_specs=in_specs,
                      out_specs=out_specs, check_rep=False),
            keep_unused=True)
        zero_shardings = tuple(
            NamedSharding(mesh, PartitionSpec("core")) for _ in range(n_outs))
        zeros = jax.jit(
            lambda: tuple(
                jnp.zeros((NHALF * a.shape[0], *a.shape[1:]), a.dtype)
                for a in out_avals),
            out_shardings=zero_shardings)()
        halves.append({"sharded": sharded, "zeros": zeros})

    return {
        "nc": nc,
        "halves": halves,
        "in_names": in_names,
        "out_idx": out_names.index("out"),
        "pool": ThreadPoolExecutor(2),
    }


_CACHE = {}


def _get_dispatch():
    if "d" not in _CACHE:
        _CACHE["d"] = _build_dispatch()
    return _CACHE["d"]


def _pull(out_j):
    for sh in out_j.addressable_shards:
        sh.data.copy_to_host_async()
    return np.asarray(out_j)


def run(inputs, trace=False, **kwargs):
    """Run on hardware; returns (full_output [B,S,C] fp32, aux)."""
    d = _get_dispatch()
    hs = inputs["hidden_states"]
    cat_w = make_w_cat(inputs["Wq"], inputs["Wk"], inputs["Wv"],
                       inputs["Wo"], inputs["bo"])
    by_name = {"wsl": cat_w}

    outs_j = []
    for hi in range(2):
        by_name["xtq"] = make_x_cat(hs, hi)
        half = d["halves"][hi]
        arrs = half["sharded"](*[by_name[n] for n in d["in_names"]],
                               *half["zeros"])
        outs_j.append(arrs[d["out_idx"]])

    # pull batch 0 in a worker thread so its download overlaps batch 1's
    # upload + exec on the full-duplex tunnel
    fut0 = d["pool"].submit(_pull, outs_j[0])
    out_g1 = _pull(outs_j[1])
    out_g0 = fut0.result()

    out = np.empty((B, S, C), np.float32)
    out[0] = out_g0.reshape(S, C)
    out[1] = out_g1.reshape(S, C)
    return out, None


def kernel(**inputs):
    out, _ = run(inputs)
    return out


# revision 16
# speedup vs baseline: 5.3588x; 1.0350x over previous
"""Trainium2 Bass kernel for diffusers AttnProcessor self-attention.

Reference computation (fp32, B=2, S=4096, C=512, H=8, D=64):
    q = hs @ Wq.T ; k = hs @ Wk.T ; v = hs @ Wv.T          (per-head split)
    probs = softmax(q k^T / sqrt(D))                        [b,h,s,s]
    out = (probs @ v) @ Wo.T + bo                           [b,s,c]

Sharding: 8 cores = (batch b in 0..1) x (query-slice of 1024 rows in 0..3).
The two batch groups are compiled as INDEPENDENT 4-core launches (meshes
over devices 0-3 and 4-7) so the axon tunnel — which is full-duplex but
only ~40MB/s each way and dominates wall-clock — can overlap batch 0's
output download with batch 1's input upload.

Each core receives ONLY its own 1024-row Xt slice plus a 1/4 slice of the
packed projection weights (+ a bo row); device-side 4-way AllGathers
rebuild the full blocked Xt and weight pack, so the host->device wire
carries each input byte exactly once per half. Output returns as fp16.

Device dataflow per core (all matmuls bf16 in / fp32 PSUM accum):
  wg = AllGather(wsl)  [4*513, C]  (wqt|wkt|wvt|wot blocks, q-scale folded,
                                    one bo row per block)
  xg = AllGather(xtq)  [4C, SQ]    (blocked Xt: block qb = Xt[:, qb*SQ:..])
  Qt = wqt^T @ xtq  per head-pair                  [128, 1024]
  Kt = wkt^T @ xg   per head-pair                  [128, 4096]
  (a per-head copy of Qt/Kt rows is DMA'd to the opposite partition half so
   the two sq-chunks of the QK^T matmul run in disjoint PE row groups)
  V' = [xg @ wvt | 1] per head                     [S, 65] per head
  per head h, per key tile t (128 keys):
    St[t] = Kt_h[:,t]^T Qt_h        [128 sk, 1024 sq]  (2 row-packed matmuls)
    Pt    = exp(St)                 (ScalarE, bf16 out)
    O'_h += V'[t]^T Pt              [65, 1024]  (row 64 = softmax denominator)
  O_h = O'_h[0:64] * (1/O'_h[64])   -> Ot (head-concat layout)
  out = Ot^T @ wot + bob            -> DMA out  [1024, 512] fp16

Dispatch: module-cached jax.jit(shard_map) closures (one per half) over the
bass_exec custom call. Output operands are device-resident dummies reused
across calls (the kernel writes every element of `out`, so their contents
never matter and they never cross the wire).
"""

import numpy as np
import ml_dtypes
from contextlib import ExitStack
from concurrent.futures import ThreadPoolExecutor

import concourse.bass as bass
import concourse.bacc as bacc
import concourse.mybir as mybir
import concourse.tile as tile

BF16 = mybir.dt.bfloat16
F16 = mybir.dt.float16
F32 = mybir.dt.float32

B, S, C, H, D = 2, 4096, 512, 8, 64
NCORES = 8
NHALF = 4          # cores per batch group / per launch
SQ = 1024          # query rows per core
P = 128            # partitions
NSK = S // P       # 32 key tiles
NCI = C // P       # 4 contraction tiles for projections
SQC = 512          # matmul moving free dim
NSQC = SQ // SQC   # 2
E = D + 1          # V' columns per head (64 v cols + ones col)
WROW = C + 1       # weight-pack rows shipped per core (one weight + bo row)

ROW_PACK = True    # run the two K=64 QK^T matmuls in disjoint PE row groups


def build_nc(row_pack=ROW_PACK, reps=1):
    # num_devices=8 with BOTH groups listed: NRT resolves a device's
    # replica group by GLOBAL device id, so one NEFF serves both 4-device
    # launches (devices 0-3 -> group 0, devices 4-7 -> group 1).
    nc = bacc.Bacc("TRN2", target_bir_lowering=False, debug=False,
                   num_devices=NCORES)

    # X ships in NATURAL [s, c] layout (a plain contiguous bf16 cast on the
    # host, ~1ms vs ~26ms for a host-side transpose); the XBAR DMA-transpose
    # produces the [c, s] SBUF tiles the matmuls need.
    xn_d = nc.dram_tensor("xn", [SQ, C], BF16, kind="ExternalInput").ap()
    wsl_d = nc.dram_tensor("wsl", [WROW, C], BF16, kind="ExternalInput").ap()
    out_d = nc.dram_tensor("out", [SQ, C], F16, kind="ExternalOutput").ap()
    # 4-core groups don't support Shared collective outputs (needs >4)
    wg_d = nc.dram_tensor("wg", [NHALF * WROW, C], BF16, kind="Internal").ap()
    xg_d = nc.dram_tensor("xg", [S, C], BF16, kind="Internal").ap()
    # collectives cannot read I/O tensors; bounce the inputs through
    # internal DRAM first
    wb_d = nc.dram_tensor("wb", [WROW, C], BF16, kind="Internal").ap()
    xb_d = nc.dram_tensor("xb", [SQ, C], BF16, kind="Internal").ap()

    with ExitStack() as ctx:
        tc = ctx.enter_context(tile.TileContext(nc))
        const = ctx.enter_context(tc.tile_pool(name="const", bufs=1))
        work = ctx.enter_context(tc.tile_pool(name="work", bufs=2))
        psum = ctx.enter_context(tc.tile_pool(name="psum", bufs=2, space="PSUM"))

        # Device-side gathers: weights first (first QK tile needs wqt+wkt),
        # then the batch group's Xt blocks. Collectives run on gpsimd in
        # issue order.
        groups = [[0, 1, 2, 3], [4, 5, 6, 7]]
        nc.gpsimd.dma_start(wb_d, wsl_d)
        nc.gpsimd.dma_start(xb_d, xn_d)
        nc.gpsimd.collective_compute(
            "AllGather", mybir.AluOpType.bypass,
            replica_groups=groups, ins=[wb_d], outs=[wg_d])
        nc.gpsimd.collective_compute(
            "AllGather", mybir.AluOpType.bypass,
            replica_groups=groups, ins=[xb_d], outs=[xg_d])

        def load_tiles(base_name, m, eng=None):
            # weight m's [C, C] block lives at gathered rows m*WROW..+C
            tiles = []
            for ci in range(NCI):
                t = const.tile([P, C], BF16, name=f"{base_name}{ci}",
                               tag=f"{base_name}{ci}")
                r = m * WROW + ci * P
                (eng or nc.sync).dma_start(t, wg_d[r:r + P, :])
                tiles.append(t)
            return tiles

        # Input loads split between the SP queue and the (startup-idle) ACT
        # queue, ordered by first use. Dependent SBUF<->SBUF moves go on the
        # gpsimd queue so they can't FIFO-block behind these.
        # own q-slice Xt tiles come straight from the (untransposed) input
        # via XBAR transpose-DMA — no gather wait
        xtq_sb = []
        for ci in range(NCI):
            t = const.tile([P, SQ], BF16, name=f"xtqs{ci}", tag=f"xtqs{ci}")
            nc.scalar.dma_start(t, xn_d[:, ci * P:(ci + 1) * P],
                                transpose=True)
            xtq_sb.append(t)
        wqt_sb = load_tiles("wqts", 0, eng=nc.scalar)
        wkt_sb = load_tiles("wkts", 1)
        xt_sb = [const.tile([P, S], BF16, name=f"xts{ci}", tag=f"xts{ci}")
                 for ci in range(NCI)]

        def load_xt_chunk(ci, ck):
            nc.sync.dma_start(
                xt_sb[ci][:, ck * SQC:(ck + 1) * SQC],
                xg_d[ck * SQC:(ck + 1) * SQC, ci * P:(ci + 1) * P],
                transpose=True)

        for ci in range(NCI):
            load_xt_chunk(ci, 0)
        wvt_sb = load_tiles("wvts", 2)
        for ck in range(1, S // SQC):
            for ci in range(NCI):
                load_xt_chunk(ci, ck)
        wot_sb = load_tiles("wots", 3)

        # bob = broadcast of bo (own input's last pack row, no gather
        # needed) to all 128 partitions via a K=1 ones-matmul
        bo1_sb = const.tile([1, C], BF16, name="bo1s", tag="bo1s")
        nc.sync.dma_start(bo1_sb, wsl_d[C:C + 1, :])
        one1_sb = const.tile([1, P], BF16, name="one1", tag="one1")
        nc.vector.memset(one1_sb, 1.0)
        bob_ps = psum.tile([P, C], F32, name="bobp", tag="proj")
        nc.tensor.matmul(bob_ps, lhsT=one1_sb, rhs=bo1_sb,
                         start=True, stop=True)
        bob_sb = const.tile([P, C], F32, name="bobs", tag="bobs")
        nc.vector.tensor_copy(out=bob_sb, in_=bob_ps)

        ones_sb = const.tile([P, D], mybir.dt.float16, name="ones_sb",
                             tag="ones_sb")
        nc.vector.memset(ones_sb, 1.0)

        for rep in range(reps):
            emit_body(nc, tc, const, work, psum, row_pack,
                      xt_sb, xtq_sb, wqt_sb, wkt_sb, wvt_sb, wot_sb,
                      bob_sb, ones_sb, out_d)

    nc.compile()
    return nc


def emit_body(nc, tc, const, work, psum, row_pack,
              xt_sb, xtq_sb, wqt_sb, wkt_sb, wvt_sb, wot_sb,
              bob_sb, ones_sb, out_d):
    vp_sb = [None] * NSK

    def emit_vproj(t_i):
        vps = psum.tile([P, C], F32, name="vps", tag="proj")
        for ci in range(NCI):
            nc.tensor.matmul(vps, lhsT=xt_sb[ci][:, t_i * P:(t_i + 1) * P],
                             rhs=wvt_sb[ci],
                             start=(ci == 0), stop=(ci == NCI - 1))
        vp = const.tile([P, H * E], BF16, name=f"vp{t_i}", tag=f"vp{t_i}")
        vp3 = vp.rearrange("p (h e) -> p h e", e=E)
        nc.vector.tensor_copy(out=vp3[:, :, 0:D],
                              in_=vps.rearrange("p (h d) -> p h d", d=D))
        nc.vector.memset(vp3[:, :, D:E], 1.0)
        vp_sb[t_i] = vp

    def emit_qtp(p):
        qtp = work.tile([P, SQ], BF16, name="qtp", tag="qtp")
        for cq in range(NSQC):
            qps = psum.tile([P, SQC], F32, name="qps", tag="proj")
            for ci in range(NCI):
                nc.tensor.matmul(
                    qps, lhsT=wqt_sb[ci][:, p * P:(p + 1) * P],
                    rhs=xtq_sb[ci][:, cq * SQC:(cq + 1) * SQC],
                    start=(ci == 0), stop=(ci == NCI - 1))
            nc.vector.tensor_copy(out=qtp[:, cq * SQC:(cq + 1) * SQC], in_=qps)
        return qtp

    def emit_ktp_chunk(ktp, p, ck):
        kps = psum.tile([P, SQC], F32, name="kps", tag="proj")
        for ci in range(NCI):
            nc.tensor.matmul(
                kps, lhsT=wkt_sb[ci][:, p * P:(p + 1) * P],
                rhs=xt_sb[ci][:, ck * SQC:(ck + 1) * SQC],
                start=(ci == 0), stop=(ci == NCI - 1))
        nc.vector.tensor_copy(out=ktp[:, ck * SQC:(ck + 1) * SQC], in_=kps)

    # Ot: normalized attention output, head-concat layout [c_in, sq]
    ot_sb = [const.tile([P, SQ], BF16, name=f"ot{i}", tag=f"ot{i}")
             for i in range(NCI)]

    def make_norm_tail(h, oraw, r):
        """Broadcast-matmul + normalize for head h. Deferred into the next
        head's loop so the PE-stream bcast matmul never waits on the DVE
        recip (PE is in-order; an early bcast would bubble the pipeline)."""
        def tail():
            rbp = psum.tile([D, SQ], F32, name="rbp", tag="st")
            for cq in range(NSQC):
                sl = slice(cq * SQC, (cq + 1) * SQC)
                nc.tensor.matmul(rbp[:, sl], lhsT=ones_sb[D:D + 1, :],
                                 rhs=r[D:D + 1, sl], start=True, stop=True)
            rb = work.tile([D, SQ], F32, name="rb", tag="rb", bufs=2)
            nc.vector.tensor_copy(out=rb, in_=rbp)
            if h % 2 == 0:
                nc.vector.tensor_mul(out=ot_sb[h // 2][0:D, :],
                                     in0=oraw[0:D, :], in1=rb)
            else:
                # DVE lanes are partition-locked; move to the upper half by DMA
                otmp = work.tile([D, SQ], BF16, name="otmp", tag="otmp",
                                 bufs=2)
                nc.vector.tensor_mul(out=otmp, in0=oraw[0:D, :], in1=rb)
                nc.gpsimd.dma_start(ot_sb[h // 2][D:2 * D, :], otmp)
        return tail

    outacc = const.tile([P, S], F16, name="outacc", tag="outacc")

    def make_oproj_tail(pair):
        """Accumulate pair `pair`'s output-projection contribution into
        outacc (SBUF). Deferred so only the final pair's slice is in the
        kernel tail."""
        def tail():
            for sqt in range(SQ // P):
                ops = psum.tile([P, C], F32, name="ops", tag="proj")
                nc.tensor.matmul(ops,
                                 lhsT=ot_sb[pair][:, sqt * P:(sqt + 1) * P],
                                 rhs=wot_sb[pair], start=True, stop=True)
                osl = outacc[:, sqt * C:(sqt + 1) * C]
                if pair == 0:
                    nc.vector.tensor_add(osl, ops, bob_sb)
                else:
                    nc.vector.tensor_add(osl, osl, ops)
            if pair == NCI - 1:
                for sqt in range(SQ // P):
                    nc.gpsimd.dma_start(
                        out_d[sqt * P:(sqt + 1) * P, :],
                        outacc[:, sqt * C:(sqt + 1) * C])
        return tail

    ktp = qtp = None
    pending_norm = None
    pending_oproj = None
    next_pair = None          # (qtp, ktp, n_chunks_pre_emitted) for pair p+1
    pre_chunks = 0
    for h in range(H):
        p, half = h // 2, h % 2
        lo, hi = half * D, half * D + D          # head's rows in pair tiles
        olo, ohi = D - half * D, 2 * D - half * D  # opposite half rows

        if half == 0:
            if next_pair is not None:
                qtp, ktp, pre_chunks = next_pair
                next_pair = None
            else:
                qtp = emit_qtp(p)
                ktp = work.tile([P, S], BF16, name="ktp", tag="ktp")
                pre_chunks = 0
        # per-head swap copies: same rows duplicated into the other
        # partition half so both sq-chunks can use disjoint PE row groups
        if row_pack:
            dma_eng = nc.gpsimd
            qts = work.tile([P, SQ], BF16, name="qts", tag="qts")
            dma_eng.dma_start(qts[olo:ohi, :], qtp[lo:hi, :])
            kts = work.tile([P, S], BF16, name="kts", tag="kts")

        def emit_k_chunk(ck):
            if half == 0 and ck >= pre_chunks:
                emit_ktp_chunk(ktp, p, ck)
            if row_pack:
                dma_eng.dma_start(
                    kts[olo:ohi, ck * SQC:(ck + 1) * SQC],
                    ktp[lo:hi, ck * SQC:(ck + 1) * SQC])

        emit_k_chunk(0)
        oacc = psum.tile([E, SQ], F32, name="oacc", tag="oacc", bufs=1)
        for t_i in range(NSK):
            # prefetch the next K chunk one window early so the QK matmuls
            # never wait on the projection->evict->swap-DMA chain
            if t_i % 4 == 1 and t_i // 4 + 1 < S // SQC:
                emit_k_chunk(t_i // 4 + 1)
            if vp_sb[t_i] is None:
                emit_vproj(t_i)
            if t_i == 8 and pending_norm is not None:
                h_prev, tail = pending_norm
                tail()
                pending_norm = None
                if h_prev % 2 == 1:
                    pending_oproj = make_oproj_tail(h_prev // 2)
            if t_i == 16 and pending_oproj is not None:
                pending_oproj()
                pending_oproj = None
            # prefetch the next pair's Q/K projections late in the second
            # head of the current pair, so the pair boundary never stalls
            # ScalarE on the projection chain
            if t_i == 24 and half == 1 and h + 1 < H and next_pair is None:
                nq = emit_qtp(p + 1)
                nk = work.tile([P, S], BF16, name="ktp", tag="ktp")
                for ck0 in range(2):
                    emit_ktp_chunk(nk, p + 1, ck0)
                next_pair = (nq, nk, 2)

            st = psum.tile([P, SQ], F32, name="st", tag="st", bufs=2)
            ksl = slice(t_i * P, (t_i + 1) * P)
            if row_pack:
                nc.tensor.matmul(st[:, 0:SQC], lhsT=ktp[lo:hi, ksl],
                                 rhs=qtp[lo:hi, 0:SQC],
                                 start=True, stop=True,
                                 tile_position=(lo, 0))
                nc.tensor.matmul(st[:, SQC:SQ], lhsT=kts[olo:ohi, ksl],
                                 rhs=qts[olo:ohi, SQC:SQ],
                                 start=True, stop=True,
                                 tile_position=(olo, 0))
            else:
                for cq in range(NSQC):
                    nc.tensor.matmul(
                        st[:, cq * SQC:(cq + 1) * SQC],
                        lhsT=ktp[lo:hi, ksl],
                        rhs=qtp[lo:hi, cq * SQC:(cq + 1) * SQC],
                        start=True, stop=True)
            pt = work.tile([P, SQ], BF16, name="pt", tag="pt", bufs=3)
            nc.scalar.activation(out=pt, in_=st,
                                 func=mybir.ActivationFunctionType.Exp)
            for cq in range(NSQC):
                nc.tensor.matmul(
                    oacc[:, cq * SQC:(cq + 1) * SQC],
                    lhsT=vp_sb[t_i][:, h * E:(h + 1) * E],
                    rhs=pt[:, cq * SQC:(cq + 1) * SQC],
                    start=(t_i == 0), stop=(t_i == NSK - 1))

        # evict oacc to SBUF immediately so the PSUM slot frees for the next
        # head; the bcast+normalize runs deferred, off the critical path
        oraw = work.tile([E, SQ], F32, name="oraw", tag="oraw", bufs=2)
        nc.vector.tensor_copy(out=oraw, in_=oacc)
        r = work.tile([E, SQ], mybir.dt.float16, name="r", tag="r", bufs=2)
        with nc.allow_low_precision("softmax denom recip; fp16 ~1e-4 rel"):
            nc.vector.reciprocal(r[D:E, :], oraw[D:E, :])
        pending_norm = (h, make_norm_tail(h, oraw, r))

    if pending_oproj is not None:      # pair 2, if heads ended before t==16
        pending_oproj()
    pending_norm[1]()                  # final head's normalization
    make_oproj_tail(NCI - 1)()         # final pair's projection + store


def make_w_cat(Wq, Wk, Wv, Wo, bo):
    """Packed weight array [4*WROW, C] bf16, shared by both halves.
    Block m = [weight m transposed (q-scale folded for m=0); bo row]."""
    bf16 = ml_dtypes.bfloat16
    scale = np.float32(D) ** -0.5
    cat_w = np.empty((NHALF * WROW, C), bf16)
    bob = np.asarray(bo, np.float32)
    for m, w in enumerate((Wq, Wk, Wv, Wo)):
        blk = cat_w[m * WROW:(m + 1) * WROW]
        wt = np.asarray(w).T
        blk[0:C] = wt * scale if m == 0 else wt
        blk[C] = bob
    return cat_w


def make_x_cat(hidden_states, b):
    """Natural-layout X input for batch b: [S, C] bf16 (contiguous cast;
    core j's shard = rows [j*SQ, (j+1)*SQ))."""
    bf16 = ml_dtypes.bfloat16
    cat_x = np.empty((S, C), bf16)
    np.copyto(cat_x, np.asarray(hidden_states[b]))
    return cat_x


def _build_dispatch():
    """Compile the Bass module and build the cached jit dispatch closures."""
    import jax
    import jax.numpy as jnp
    from jax.sharding import Mesh, PartitionSpec, NamedSharding
    from jax.experimental.shard_map import shard_map
    from concourse.bass2jax import (
        _bass_exec_p, partition_id_tensor, install_neuronx_cc_hook)

    nc = build_nc()
    install_neuronx_cc_hook()
    assert nc.dbg_addr is None, "build with debug=False"

    partition_name = (nc.partition_id_tensor.name
                      if nc.partition_id_tensor else None)
    in_names, out_names, out_avals = [], [], []
    for alloc in nc.m.functions[0].allocations:
        if not isinstance(alloc, mybir.MemoryLocationSet):
            continue
        name = alloc.memorylocations[0].name
        if alloc.kind == "ExternalInput":
            if name != partition_name:
                in_names.append(name)
        elif alloc.kind == "ExternalOutput":
            shape = tuple(alloc.tensor_shape)
            dtype = mybir.dt.np(alloc.dtype)
            out_names.append(name)
            out_avals.append(jax.core.ShapedArray(shape, dtype))
    n_params = len(in_names)
    n_outs = len(out_avals)
    all_names = in_names + out_names + (
        [partition_name] if partition_name else [])

    def _body(*args):
        operands = list(args)
        if partition_name is not None:
            operands.append(partition_id_tensor())
        outs = _bass_exec_p.bind(
            *operands,
            out_avals=tuple(out_avals),
            in_names=tuple(all_names),
            out_names=tuple(out_names),
            lowering_input_output_aliases=(),
            sim_require_finite=True,
            sim_require_nnan=True,
            nc=nc,
        )
        return tuple(outs)

    devices = jax.devices()[:NCORES]
    halves = []
    for hi in range(2):
        mesh = Mesh(np.asarray(devices[hi * NHALF:(hi + 1) * NHALF]),
                    ("core",))
        in_specs = (PartitionSpec("core"),) * (n_params + n_outs)
        out_specs = (PartitionSpec("core"),) * n_outs
        # No donation: the kernel writes every element of `out`, so the
        # output operands are never read and can be device-resident dummies
        # reused across calls (zero wire traffic, zero per-call work).
        sharded = jax.jit(
            shard_map(_body, mesh=mesh, in_specs=in_specs,
                      out_specs=out_specs, check_rep=False),
            keep_unused=True)
        zero_shardings = tuple(
            NamedSharding(mesh, PartitionSpec("core")) for _ in range(n_outs))
        zeros = jax.jit(
            lambda: tuple(
                jnp.zeros((NHALF * a.shape[0], *a.shape[1:]), a.dtype)
                for a in out_avals),
            out_shardings=zero_shardings)()
        halves.append({"sharded": sharded, "zeros": zeros})

    return {
        "nc": nc,
        "halves": halves,
        "in_names": in_names,
        "out_idx": out_names.index("out"),
        "pool": ThreadPoolExecutor(2),
    }


_CACHE = {}


def _get_dispatch():
    if "d" not in _CACHE:
        _CACHE["d"] = _build_dispatch()
    return _CACHE["d"]


def _pull(out_j):
    for sh in out_j.addressable_shards:
        sh.data.copy_to_host_async()
    return np.asarray(out_j)


def run(inputs, trace=False, **kwargs):
    """Run on hardware; returns (full_output [B,S,C] fp32, aux)."""
    d = _get_dispatch()
    hs = inputs["hidden_states"]
    cat_w = make_w_cat(inputs["Wq"], inputs["Wk"], inputs["Wv"],
                       inputs["Wo"], inputs["bo"])
    by_name = {"wsl": cat_w}

    outs_j = []
    for hi in range(2):
        by_name["xn"] = make_x_cat(hs, hi)
        half = d["halves"][hi]
        arrs = half["sharded"](*[by_name[n] for n in d["in_names"]],
                               *half["zeros"])
        outs_j.append(arrs[d["out_idx"]])

    # pull batch 0 in a worker thread so its download overlaps batch 1's
    # upload + exec on the full-duplex tunnel
    fut0 = d["pool"].submit(_pull, outs_j[0])
    out_g1 = _pull(outs_j[1])
    out_g0 = fut0.result()

    out = np.empty((B, S, C), np.float32)
    out[0] = out_g0.reshape(S, C)
    out[1] = out_g1.reshape(S, C)
    return out, None


def kernel(**inputs):
    out, _ = run(inputs)
    return out


# revision 22
# speedup vs baseline: 5.7434x; 1.0718x over previous
"""Trainium2 Bass kernel for diffusers AttnProcessor self-attention.

Reference computation (fp32, B=2, S=4096, C=512, H=8, D=64):
    q = hs @ Wq.T ; k = hs @ Wk.T ; v = hs @ Wv.T          (per-head split)
    probs = softmax(q k^T / sqrt(D))                        [b,h,s,s]
    out = (probs @ v) @ Wo.T + bo                           [b,s,c]

Sharding: 8 cores = (batch b in 0..1) x (query-slice of 1024 rows in 0..3).
The two batch groups are compiled as INDEPENDENT 4-core launches (meshes
over devices 0-3 and 4-7) so the axon tunnel — which is full-duplex but
only ~40MB/s each way and dominates wall-clock — can overlap batch 0's
output download with batch 1's input upload.

Each core receives ONLY its own 1024-row Xt slice plus a 1/4 slice of the
packed projection weights (+ a bo row); device-side 4-way AllGathers
rebuild the full blocked Xt and weight pack, so the host->device wire
carries each input byte exactly once per half. Output returns as fp16.

Device dataflow per core (all matmuls bf16 in / fp32 PSUM accum):
  wg = AllGather(wsl)  [4*513, C]  (wqt|wkt|wvt|wot blocks, q-scale folded,
                                    one bo row per block)
  xg = AllGather(xtq)  [4C, SQ]    (blocked Xt: block qb = Xt[:, qb*SQ:..])
  Qt = wqt^T @ xtq  per head-pair                  [128, 1024]
  Kt = wkt^T @ xg   per head-pair                  [128, 4096]
  (a per-head copy of Qt/Kt rows is DMA'd to the opposite partition half so
   the two sq-chunks of the QK^T matmul run in disjoint PE row groups)
  V' = [xg @ wvt | 1] per head                     [S, 65] per head
  per head h, per key tile t (128 keys):
    St[t] = Kt_h[:,t]^T Qt_h        [128 sk, 1024 sq]  (2 row-packed matmuls)
    Pt    = exp(St)                 (ScalarE, bf16 out)
    O'_h += V'[t]^T Pt              [65, 1024]  (row 64 = softmax denominator)
  O_h = O'_h[0:64] * (1/O'_h[64])   -> Ot (head-concat layout)
  out = Ot^T @ wot + bob            -> DMA out  [1024, 512] fp16

Dispatch: module-cached jax.jit(shard_map) closures (one per half) over the
bass_exec custom call. Output operands are device-resident dummies reused
across calls (the kernel writes every element of `out`, so their contents
never matter and they never cross the wire).
"""

import numpy as np
import ml_dtypes
from contextlib import ExitStack
from concurrent.futures import ThreadPoolExecutor

import concourse.bass as bass
import concourse.bacc as bacc
import concourse.mybir as mybir
import concourse.tile as tile

BF16 = mybir.dt.bfloat16
F16 = mybir.dt.float16
F32 = mybir.dt.float32
I8 = mybir.dt.int8

B, S, C, H, D = 2, 4096, 512, 8, 64
NCORES = 8
NHALF = 4          # cores per batch group / per launch
SQ = 1024          # query rows per core
P = 128            # partitions
NSK = S // P       # 32 key tiles
NCI = C // P       # 4 contraction tiles for projections
SQC = 512          # matmul moving free dim
NSQC = SQ // SQC   # 2
E = D + 1          # V' columns per head (64 v cols + ones col)
WROW = C + 1       # weight-pack rows shipped per core (one weight + bo row)

ROW_PACK = True    # run the two K=64 QK^T matmuls in disjoint PE row groups


def build_nc(row_pack=ROW_PACK, reps=1):
    # num_devices=8 with BOTH groups listed: NRT resolves a device's
    # replica group by GLOBAL device id, so one NEFF serves both 4-device
    # launches (devices 0-3 -> group 0, devices 4-7 -> group 1).
    nc = bacc.Bacc("TRN2", target_bir_lowering=False, debug=False,
                   num_devices=NCORES)

    # X ships int8 row-quantized in NATURAL [s, c] layout (halves the upload
    # vs bf16; rel err ~8e-3 vs the 2e-2 budget). The device dequantizes to
    # bf16 (per-row scales, ~15us) and the XBAR DMA-transpose produces the
    # [c, s] SBUF tiles the matmuls need.
    xq_d = nc.dram_tensor("xq", [SQ, C], I8, kind="ExternalInput").ap()
    xs_d = nc.dram_tensor("xs", [SQ, 1], F32, kind="ExternalInput").ap()
    wsl_d = nc.dram_tensor("wsl", [WROW, C], BF16, kind="ExternalInput").ap()
    out_d = nc.dram_tensor("out", [SQ, C], F16, kind="ExternalOutput").ap()
    # 4-core groups don't support Shared collective outputs (needs >4)
    wg_d = nc.dram_tensor("wg", [NHALF * WROW, C], BF16, kind="Internal").ap()
    xg_d = nc.dram_tensor("xg", [S, C], BF16, kind="Internal").ap()
    # dequantized own slice: collective input (collectives cannot read I/O
    # tensors) and XBAR source for the own-q-slice Xt tiles
    xnb_d = nc.dram_tensor("xnb", [SQ, C], BF16, kind="Internal").ap()
    wb_d = nc.dram_tensor("wb", [WROW, C], BF16, kind="Internal").ap()

    with ExitStack() as ctx:
        tc = ctx.enter_context(tile.TileContext(nc))
        const = ctx.enter_context(tc.tile_pool(name="const", bufs=1))
        work = ctx.enter_context(tc.tile_pool(name="work", bufs=2))
        psum = ctx.enter_context(tc.tile_pool(name="psum", bufs=2, space="PSUM"))

        # Device-side gathers: weights first (first QK tile needs wqt+wkt),
        # then the batch group's Xt blocks. Collectives run on gpsimd in
        # issue order.
        groups = [[0, 1, 2, 3], [4, 5, 6, 7]]
        nc.gpsimd.dma_start(wb_d, wsl_d)
        # dequantize the own X slice to bf16 (per-row scale on partitions)
        for si in range(SQ // P):
            sl = slice(si * P, (si + 1) * P)
            xqt = work.tile([P, C], I8, name="xqt", tag="xqt", bufs=2)
            nc.sync.dma_start(xqt, xq_d[sl, :])
            xst = work.tile([P, 1], F32, name="xst", tag="xst", bufs=2)
            nc.sync.dma_start(xst, xs_d[sl, :])
            xbt = work.tile([P, C], BF16, name="xbt", tag="xbt", bufs=2)
            nc.vector.tensor_scalar_mul(xbt, xqt, xst)
            nc.gpsimd.dma_start(xnb_d[sl, :], xbt)
        nc.gpsimd.collective_compute(
            "AllGather", mybir.AluOpType.bypass,
            replica_groups=groups, ins=[wb_d], outs=[wg_d])
        nc.gpsimd.collective_compute(
            "AllGather", mybir.AluOpType.bypass,
            replica_groups=groups, ins=[xnb_d], outs=[xg_d])

        def load_tiles(base_name, m, eng=None):
            # weight m's [C, C] block lives at gathered rows m*WROW..+C
            tiles = []
            for ci in range(NCI):
                t = const.tile([P, C], BF16, name=f"{base_name}{ci}",
                               tag=f"{base_name}{ci}")
                r = m * WROW + ci * P
                (eng or nc.sync).dma_start(t, wg_d[r:r + P, :])
                tiles.append(t)
            return tiles

        # Input loads split between the SP queue and the (startup-idle) ACT
        # queue, ordered by first use. Dependent SBUF<->SBUF moves go on the
        # gpsimd queue so they can't FIFO-block behind these.
        # own q-slice Xt tiles come from the dequantized own slice via XBAR
        # transpose-DMA — no gather wait
        xtq_sb = []
        for ci in range(NCI):
            t = const.tile([P, SQ], BF16, name=f"xtqs{ci}", tag=f"xtqs{ci}")
            nc.scalar.dma_start(t, xnb_d[:, ci * P:(ci + 1) * P],
                                transpose=True)
            xtq_sb.append(t)
        wqt_sb = load_tiles("wqts", 0, eng=nc.scalar)
        wkt_sb = load_tiles("wkts", 1)
        xt_sb = [const.tile([P, S], BF16, name=f"xts{ci}", tag=f"xts{ci}")
                 for ci in range(NCI)]

        def load_xt_chunk(ci, ck):
            nc.sync.dma_start(
                xt_sb[ci][:, ck * SQC:(ck + 1) * SQC],
                xg_d[ck * SQC:(ck + 1) * SQC, ci * P:(ci + 1) * P],
                transpose=True)

        for ci in range(NCI):
            load_xt_chunk(ci, 0)
        wvt_sb = load_tiles("wvts", 2)
        for ck in range(1, S // SQC):
            for ci in range(NCI):
                load_xt_chunk(ci, ck)
        wot_sb = load_tiles("wots", 3)

        # bob = broadcast of bo (own input's last pack row, no gather
        # needed) to all 128 partitions via a K=1 ones-matmul
        bo1_sb = const.tile([1, C], BF16, name="bo1s", tag="bo1s")
        nc.sync.dma_start(bo1_sb, wsl_d[C:C + 1, :])
        one1_sb = const.tile([1, P], BF16, name="one1", tag="one1")
        nc.vector.memset(one1_sb, 1.0)
        bob_ps = psum.tile([P, C], F32, name="bobp", tag="proj")
        nc.tensor.matmul(bob_ps, lhsT=one1_sb, rhs=bo1_sb,
                         start=True, stop=True)
        bob_sb = const.tile([P, C], F32, name="bobs", tag="bobs")
        nc.vector.tensor_copy(out=bob_sb, in_=bob_ps)

        ones_sb = const.tile([P, D], mybir.dt.float16, name="ones_sb",
                             tag="ones_sb")
        nc.vector.memset(ones_sb, 1.0)

        for rep in range(reps):
            emit_body(nc, tc, const, work, psum, row_pack,
                      xt_sb, xtq_sb, wqt_sb, wkt_sb, wvt_sb, wot_sb,
                      bob_sb, ones_sb, out_d)

    nc.compile()
    return nc


def emit_body(nc, tc, const, work, psum, row_pack,
              xt_sb, xtq_sb, wqt_sb, wkt_sb, wvt_sb, wot_sb,
              bob_sb, ones_sb, out_d):
    vp_sb = [None] * NSK

    def emit_vproj(t_i):
        vps = psum.tile([P, C], F32, name="vps", tag="proj")
        for ci in range(NCI):
            nc.tensor.matmul(vps, lhsT=xt_sb[ci][:, t_i * P:(t_i + 1) * P],
                             rhs=wvt_sb[ci],
                             start=(ci == 0), stop=(ci == NCI - 1))
        vp = const.tile([P, H * E], BF16, name=f"vp{t_i}", tag=f"vp{t_i}")
        vp3 = vp.rearrange("p (h e) -> p h e", e=E)
        nc.vector.tensor_copy(out=vp3[:, :, 0:D],
                              in_=vps.rearrange("p (h d) -> p h d", d=D))
        nc.vector.memset(vp3[:, :, D:E], 1.0)
        vp_sb[t_i] = vp

    def emit_qtp(p):
        qtp = work.tile([P, SQ], BF16, name="qtp", tag="qtp")
        for cq in range(NSQC):
            qps = psum.tile([P, SQC], F32, name="qps", tag="proj")
            for ci in range(NCI):
                nc.tensor.matmul(
                    qps, lhsT=wqt_sb[ci][:, p * P:(p + 1) * P],
                    rhs=xtq_sb[ci][:, cq * SQC:(cq + 1) * SQC],
                    start=(ci == 0), stop=(ci == NCI - 1))
            nc.vector.tensor_copy(out=qtp[:, cq * SQC:(cq + 1) * SQC], in_=qps)
        return qtp

    def emit_ktp_chunk(ktp, p, ck):
        kps = psum.tile([P, SQC], F32, name="kps", tag="proj")
        for ci in range(NCI):
            nc.tensor.matmul(
                kps, lhsT=wkt_sb[ci][:, p * P:(p + 1) * P],
                rhs=xt_sb[ci][:, ck * SQC:(ck + 1) * SQC],
                start=(ci == 0), stop=(ci == NCI - 1))
        nc.vector.tensor_copy(out=ktp[:, ck * SQC:(ck + 1) * SQC], in_=kps)

    # Ot: normalized attention output, head-concat layout [c_in, sq]
    ot_sb = [const.tile([P, SQ], BF16, name=f"ot{i}", tag=f"ot{i}")
             for i in range(NCI)]

    def make_norm_tail(h, oraw, r):
        """Broadcast-matmul + normalize for head h. Deferred into the next
        head's loop so the PE-stream bcast matmul never waits on the DVE
        recip (PE is in-order; an early bcast would bubble the pipeline)."""
        def tail():
            rbp = psum.tile([D, SQ], F32, name="rbp", tag="st")
            for cq in range(NSQC):
                sl = slice(cq * SQC, (cq + 1) * SQC)
                nc.tensor.matmul(rbp[:, sl], lhsT=ones_sb[D:D + 1, :],
                                 rhs=r[D:D + 1, sl], start=True, stop=True)
            rb = work.tile([D, SQ], F32, name="rb", tag="rb", bufs=2)
            nc.vector.tensor_copy(out=rb, in_=rbp)
            if h % 2 == 0:
                nc.vector.tensor_mul(out=ot_sb[h // 2][0:D, :],
                                     in0=oraw[0:D, :], in1=rb)
            else:
                # DVE lanes are partition-locked; move to the upper half by DMA
                otmp = work.tile([D, SQ], BF16, name="otmp", tag="otmp",
                                 bufs=2)
                nc.vector.tensor_mul(out=otmp, in0=oraw[0:D, :], in1=rb)
                nc.gpsimd.dma_start(ot_sb[h // 2][D:2 * D, :], otmp)
        return tail

    outacc = const.tile([P, S], F16, name="outacc", tag="outacc")

    def make_oproj_tail(pair):
        """Accumulate pair `pair`'s output-projection contribution into
        outacc (SBUF). Deferred so only the final pair's slice is in the
        kernel tail."""
        def tail():
            for sqt in range(SQ // P):
                ops = psum.tile([P, C], F32, name="ops", tag="proj")
                nc.tensor.matmul(ops,
                                 lhsT=ot_sb[pair][:, sqt * P:(sqt + 1) * P],
                                 rhs=wot_sb[pair], start=True, stop=True)
                osl = outacc[:, sqt * C:(sqt + 1) * C]
                if pair == 0:
                    nc.vector.tensor_add(osl, ops, bob_sb)
                else:
                    nc.vector.tensor_add(osl, osl, ops)
            if pair == NCI - 1:
                for sqt in range(SQ // P):
                    nc.gpsimd.dma_start(
                        out_d[sqt * P:(sqt + 1) * P, :],
                        outacc[:, sqt * C:(sqt + 1) * C])
        return tail

    ktp = qtp = None
    pending_norm = None
    pending_oproj = None
    next_pair = None          # (qtp, ktp, n_chunks_pre_emitted) for pair p+1
    pre_chunks = 0
    for h in range(H):
        p, half = h // 2, h % 2
        lo, hi = half * D, half * D + D          # head's rows in pair tiles
        olo, ohi = D - half * D, 2 * D - half * D  # opposite half rows

        if half == 0:
            if next_pair is not None:
                qtp, ktp, pre_chunks = next_pair
                next_pair = None
            else:
                qtp = emit_qtp(p)
                ktp = work.tile([P, S], BF16, name="ktp", tag="ktp")
                pre_chunks = 0
        # per-head swap copies: same rows duplicated into the other
        # partition half so both sq-chunks can use disjoint PE row groups
        if row_pack:
            dma_eng = nc.gpsimd
            qts = work.tile([P, SQ], BF16, name="qts", tag="qts")
            dma_eng.dma_start(qts[olo:ohi, :], qtp[lo:hi, :])
            kts = work.tile([P, S], BF16, name="kts", tag="kts")

        def emit_k_chunk(ck):
            if half == 0 and ck >= pre_chunks:
                emit_ktp_chunk(ktp, p, ck)
            if row_pack:
                dma_eng.dma_start(
                    kts[olo:ohi, ck * SQC:(ck + 1) * SQC],
                    ktp[lo:hi, ck * SQC:(ck + 1) * SQC])

        emit_k_chunk(0)
        oacc = psum.tile([E, SQ], F32, name="oacc", tag="oacc", bufs=1)
        for t_i in range(NSK):
            # prefetch the next K chunk one window early so the QK matmuls
            # never wait on the projection->evict->swap-DMA chain
            if t_i % 4 == 1 and t_i // 4 + 1 < S // SQC:
                emit_k_chunk(t_i // 4 + 1)
            if vp_sb[t_i] is None:
                emit_vproj(t_i)
            if t_i == 8 and pending_norm is not None:
                h_prev, tail = pending_norm
                tail()
                pending_norm = None
                if h_prev % 2 == 1:
                    pending_oproj = make_oproj_tail(h_prev // 2)
            if t_i == 16 and pending_oproj is not None:
                pending_oproj()
                pending_oproj = None
            # prefetch the next pair's Q/K projections late in the second
            # head of the current pair, so the pair boundary never stalls
            # ScalarE on the projection chain
            if t_i == 24 and half == 1 and h + 1 < H and next_pair is None:
                nq = emit_qtp(p + 1)
                nk = work.tile([P, S], BF16, name="ktp", tag="ktp")
                for ck0 in range(2):
                    emit_ktp_chunk(nk, p + 1, ck0)
                next_pair = (nq, nk, 2)

            st = psum.tile([P, SQ], F32, name="st", tag="st", bufs=2)
            ksl = slice(t_i * P, (t_i + 1) * P)
            if row_pack:
                nc.tensor.matmul(st[:, 0:SQC], lhsT=ktp[lo:hi, ksl],
                                 rhs=qtp[lo:hi, 0:SQC],
                                 start=True, stop=True,
                                 tile_position=(lo, 0))
                nc.tensor.matmul(st[:, SQC:SQ], lhsT=kts[olo:ohi, ksl],
                                 rhs=qts[olo:ohi, SQC:SQ],
                                 start=True, stop=True,
                                 tile_position=(olo, 0))
            else:
                for cq in range(NSQC):
                    nc.tensor.matmul(
                        st[:, cq * SQC:(cq + 1) * SQC],
                        lhsT=ktp[lo:hi, ksl],
                        rhs=qtp[lo:hi, cq * SQC:(cq + 1) * SQC],
                        start=True, stop=True)
            pt = work.tile([P, SQ], BF16, name="pt", tag="pt", bufs=3)
            nc.scalar.activation(out=pt, in_=st,
                                 func=mybir.ActivationFunctionType.Exp)
            for cq in range(NSQC):
                nc.tensor.matmul(
                    oacc[:, cq * SQC:(cq + 1) * SQC],
                    lhsT=vp_sb[t_i][:, h * E:(h + 1) * E],
                    rhs=pt[:, cq * SQC:(cq + 1) * SQC],
                    start=(t_i == 0), stop=(t_i == NSK - 1))

        # evict oacc to SBUF immediately so the PSUM slot frees for the next
        # head; the bcast+normalize runs deferred, off the critical path
        oraw = work.tile([E, SQ], F32, name="oraw", tag="oraw", bufs=2)
        nc.vector.tensor_copy(out=oraw, in_=oacc)
        r = work.tile([E, SQ], mybir.dt.float16, name="r", tag="r", bufs=2)
        with nc.allow_low_precision("softmax denom recip; fp16 ~1e-4 rel"):
            nc.vector.reciprocal(r[D:E, :], oraw[D:E, :])
        pending_norm = (h, make_norm_tail(h, oraw, r))

    if pending_oproj is not None:      # pair 2, if heads ended before t==16
        pending_oproj()
    pending_norm[1]()                  # final head's normalization
    make_oproj_tail(NCI - 1)()         # final pair's projection + store


def make_w_cat(Wq, Wk, Wv, Wo, bo):
    """Packed weight array [4*WROW, C] bf16, shared by both halves.
    Block m = [weight m transposed (q-scale folded for m=0); bo row]."""
    bf16 = ml_dtypes.bfloat16
    scale = np.float32(D) ** -0.5
    cat_w = np.empty((NHALF * WROW, C), bf16)
    bob = np.asarray(bo, np.float32)
    for m, w in enumerate((Wq, Wk, Wv, Wo)):
        blk = cat_w[m * WROW:(m + 1) * WROW]
        wt = np.asarray(w).T
        blk[0:C] = wt * scale if m == 0 else wt
        blk[C] = bob
    return cat_w


def make_x_cat(hidden_states, b):
    """int8 row-quantized X for batch b: ([S, C] int8, [S, 1] f32 scales).
    Core j's shard = rows [j*SQ, (j+1)*SQ)."""
    xb = np.asarray(hidden_states[b])
    m = np.abs(xb).max(axis=1, keepdims=True)
    s = np.maximum(m, np.float32(1e-20)) * np.float32(1 / 127.0)
    xq = np.rint(xb * (np.float32(1.0) / s)).astype(np.int8)
    return xq, s.astype(np.float32)


def _build_dispatch():
    """Compile the Bass module and build the cached jit dispatch closures."""
    import jax
    import jax.numpy as jnp
    from jax.sharding import Mesh, PartitionSpec, NamedSharding
    from jax.experimental.shard_map import shard_map
    from concourse.bass2jax import (
        _bass_exec_p, partition_id_tensor, install_neuronx_cc_hook)

    nc = build_nc()
    install_neuronx_cc_hook()
    assert nc.dbg_addr is None, "build with debug=False"

    partition_name = (nc.partition_id_tensor.name
                      if nc.partition_id_tensor else None)
    in_names, out_names, out_avals = [], [], []
    for alloc in nc.m.functions[0].allocations:
        if not isinstance(alloc, mybir.MemoryLocationSet):
            continue
        name = alloc.memorylocations[0].name
        if alloc.kind == "ExternalInput":
            if name != partition_name:
                in_names.append(name)
        elif alloc.kind == "ExternalOutput":
            shape = tuple(alloc.tensor_shape)
            dtype = mybir.dt.np(alloc.dtype)
            out_names.append(name)
            out_avals.append(jax.core.ShapedArray(shape, dtype))
    n_params = len(in_names)
    n_outs = len(out_avals)
    all_names = in_names + out_names + (
        [partition_name] if partition_name else [])

    def _body(*args):
        operands = list(args)
        if partition_name is not None:
            operands.append(partition_id_tensor())
        outs = _bass_exec_p.bind(
            *operands,
            out_avals=tuple(out_avals),
            in_names=tuple(all_names),
            out_names=tuple(out_names),
            lowering_input_output_aliases=(),
            sim_require_finite=True,
            sim_require_nnan=True,
            nc=nc,
        )
        return tuple(outs)

    devices = jax.devices()[:NCORES]
    halves = []
    for hi in range(2):
        mesh = Mesh(np.asarray(devices[hi * NHALF:(hi + 1) * NHALF]),
                    ("core",))
        in_specs = (PartitionSpec("core"),) * (n_params + n_outs)
        out_specs = (PartitionSpec("core"),) * n_outs
        # No donation: the kernel writes every element of `out`, so the
        # output operands are never read and can be device-resident dummies
        # reused across calls (zero wire traffic, zero per-call work).
        sharded = jax.jit(
            shard_map(_body, mesh=mesh, in_specs=in_specs,
                      out_specs=out_specs, check_rep=False),
            keep_unused=True)
        zero_shardings = tuple(
            NamedSharding(mesh, PartitionSpec("core")) for _ in range(n_outs))
        zeros = jax.jit(
            lambda: tuple(
                jnp.zeros((NHALF * a.shape[0], *a.shape[1:]), a.dtype)
                for a in out_avals),
            out_shardings=zero_shardings)()
        halves.append({"sharded": sharded, "zeros": zeros})

    return {
        "nc": nc,
        "halves": halves,
        "in_names": in_names,
        "out_idx": out_names.index("out"),
        "pool": ThreadPoolExecutor(2),
    }


_CACHE = {}


def _get_dispatch():
    if "d" not in _CACHE:
        _CACHE["d"] = _build_dispatch()
    return _CACHE["d"]


def _pull(out_j):
    for sh in out_j.addressable_shards:
        sh.data.copy_to_host_async()
    return np.asarray(out_j)


def run(inputs, trace=False, **kwargs):
    """Run on hardware; returns (full_output [B,S,C] fp32, aux)."""
    d = _get_dispatch()
    hs = inputs["hidden_states"]
    cat_w = make_w_cat(inputs["Wq"], inputs["Wk"], inputs["Wv"],
                       inputs["Wo"], inputs["bo"])
    by_name = {"wsl": cat_w}

    outs_j = []
    for hi in range(2):
        by_name["xq"], by_name["xs"] = make_x_cat(hs, hi)
        half = d["halves"][hi]
        arrs = half["sharded"](*[by_name[n] for n in d["in_names"]],
                               *half["zeros"])
        outs_j.append(arrs[d["out_idx"]])

    # pull batch 0 in a worker thread so its download overlaps batch 1's
    # upload + exec on the full-duplex tunnel
    fut0 = d["pool"].submit(_pull, outs_j[0])
    out_g1 = _pull(outs_j[1])
    out_g0 = fut0.result()

    out = np.empty((B, S, C), np.float32)
    out[0] = out_g0.reshape(S, C)
    out[1] = out_g1.reshape(S, C)
    return out, None


def kernel(**inputs):
    out, _ = run(inputs)
    return out


# revision 28
# speedup vs baseline: 7.6105x; 1.3251x over previous
"""Trainium2 Bass kernel for diffusers AttnProcessor self-attention.

Reference computation (fp32, B=2, S=4096, C=512, H=8, D=64):
    q = hs @ Wq.T ; k = hs @ Wk.T ; v = hs @ Wv.T          (per-head split)
    probs = softmax(q k^T / sqrt(D))                        [b,h,s,s]
    out = (probs @ v) @ Wo.T + bo                           [b,s,c]

Sharding: 8 cores = (batch b in 0..1) x (query-slice of 1024 rows in 0..3).
The two batch groups are compiled as INDEPENDENT 4-core launches (meshes
over devices 0-3 and 4-7) so the axon tunnel — which is full-duplex but
only ~40MB/s each way and dominates wall-clock — can overlap batch 0's
output download with batch 1's input upload.

Each core receives ONLY its own 1024-row Xt slice plus a 1/4 slice of the
packed projection weights (+ a bo row); device-side 4-way AllGathers
rebuild the full blocked Xt and weight pack, so the host->device wire
carries each input byte exactly once per half. Output returns as fp16.

Device dataflow per core (all matmuls bf16 in / fp32 PSUM accum):
  wg = AllGather(wsl)  [4*513, C]  (wqt|wkt|wvt|wot blocks, q-scale folded,
                                    one bo row per block)
  xg = AllGather(xtq)  [4C, SQ]    (blocked Xt: block qb = Xt[:, qb*SQ:..])
  Qt = wqt^T @ xtq  per head-pair                  [128, 1024]
  Kt = wkt^T @ xg   per head-pair                  [128, 4096]
  (a per-head copy of Qt/Kt rows is DMA'd to the opposite partition half so
   the two sq-chunks of the QK^T matmul run in disjoint PE row groups)
  V' = [xg @ wvt | 1] per head                     [S, 65] per head
  per head h, per key tile t (128 keys):
    St[t] = Kt_h[:,t]^T Qt_h        [128 sk, 1024 sq]  (2 row-packed matmuls)
    Pt    = exp(St)                 (ScalarE, bf16 out)
    O'_h += V'[t]^T Pt              [65, 1024]  (row 64 = softmax denominator)
  O_h = O'_h[0:64] * (1/O'_h[64])   -> Ot (head-concat layout)
  out = Ot^T @ wot + bob            -> DMA out  [1024, 512] fp16

Dispatch: module-cached jax.jit(shard_map) closures (one per half) over the
bass_exec custom call. Output operands are device-resident dummies reused
across calls (the kernel writes every element of `out`, so their contents
never matter and they never cross the wire).
"""

import numpy as np
import ml_dtypes
from contextlib import ExitStack
from concurrent.futures import ThreadPoolExecutor

import concourse.bass as bass
import concourse.bacc as bacc
import concourse.mybir as mybir
import concourse.tile as tile

BF16 = mybir.dt.bfloat16
F16 = mybir.dt.float16
F32 = mybir.dt.float32
I8 = mybir.dt.int8

B, S, C, H, D = 2, 4096, 512, 8, 64
NCORES = 8
NHALF = 4          # cores per batch group / per launch
SQ = 1024          # query rows per core
P = 128            # partitions
NSK = S // P       # 32 key tiles
NCI = C // P       # 4 contraction tiles for projections
SQC = 512          # matmul moving free dim
NSQC = SQ // SQC   # 2
E = D + 1          # V' columns per head (64 v cols + ones col)
WROW = C + 1       # weight-pack rows shipped per core (one weight + bo row)

ROW_PACK = True    # run the two K=64 QK^T matmuls in disjoint PE row groups


def build_nc(row_pack=ROW_PACK, reps=1):
    # num_devices=8 with BOTH groups listed: NRT resolves a device's
    # replica group by GLOBAL device id, so one NEFF serves both 4-device
    # launches (devices 0-3 -> group 0, devices 4-7 -> group 1).
    nc = bacc.Bacc("TRN2", target_bir_lowering=False, debug=False,
                   num_devices=NCORES)

    # X ships int8 row-quantized in NATURAL [s, c] layout (halves the upload
    # vs bf16; rel err ~8e-3 vs the 2e-2 budget). The device dequantizes to
    # bf16 (per-row scales, ~15us) and the XBAR DMA-transpose produces the
    # [c, s] SBUF tiles the matmuls need.
    xq_d = nc.dram_tensor("xq", [SQ, C], I8, kind="ExternalInput").ap()
    xs_d = nc.dram_tensor("xs", [SQ, 1], F32, kind="ExternalInput").ap()
    wsl_d = nc.dram_tensor("wsl", [WROW, C], BF16, kind="ExternalInput").ap()
    # output also ships int8 row-quantized (+ per-row f32 scales)
    outq_d = nc.dram_tensor("outq", [SQ, C], I8, kind="ExternalOutput").ap()
    outs_d = nc.dram_tensor("outs", [SQ, 1], F32, kind="ExternalOutput").ap()
    # 4-core groups don't support Shared collective outputs (needs >4)
    wg_d = nc.dram_tensor("wg", [NHALF * WROW, C], BF16, kind="Internal").ap()
    xg_d = nc.dram_tensor("xg", [S, C], BF16, kind="Internal").ap()
    # dequantized own slice: collective input (collectives cannot read I/O
    # tensors) and XBAR source for the own-q-slice Xt tiles
    xnb_d = nc.dram_tensor("xnb", [SQ, C], BF16, kind="Internal").ap()
    wb_d = nc.dram_tensor("wb", [WROW, C], BF16, kind="Internal").ap()

    with ExitStack() as ctx:
        tc = ctx.enter_context(tile.TileContext(nc))
        const = ctx.enter_context(tc.tile_pool(name="const", bufs=1))
        work = ctx.enter_context(tc.tile_pool(name="work", bufs=2))
        psum = ctx.enter_context(tc.tile_pool(name="psum", bufs=2, space="PSUM"))

        # Device-side gathers: weights first (first QK tile needs wqt+wkt),
        # then the batch group's Xt blocks. Collectives run on gpsimd in
        # issue order.
        groups = [[0, 1, 2, 3], [4, 5, 6, 7]]
        nc.gpsimd.dma_start(wb_d, wsl_d)
        # dequantize the own X slice to bf16 (per-row scale on partitions)
        for si in range(SQ // P):
            sl = slice(si * P, (si + 1) * P)
            xqt = work.tile([P, C], I8, name="xqt", tag="xqt", bufs=2)
            nc.sync.dma_start(xqt, xq_d[sl, :])
            xst = work.tile([P, 1], F32, name="xst", tag="xst", bufs=2)
            nc.sync.dma_start(xst, xs_d[sl, :])
            xbt = work.tile([P, C], BF16, name="xbt", tag="xbt", bufs=2)
            nc.vector.tensor_scalar_mul(xbt, xqt, xst)
            nc.gpsimd.dma_start(xnb_d[sl, :], xbt)
        nc.gpsimd.collective_compute(
            "AllGather", mybir.AluOpType.bypass,
            replica_groups=groups, ins=[wb_d], outs=[wg_d])
        nc.gpsimd.collective_compute(
            "AllGather", mybir.AluOpType.bypass,
            replica_groups=groups, ins=[xnb_d], outs=[xg_d])

        def load_tiles(base_name, m, eng=None):
            # weight m's [C, C] block lives at gathered rows m*WROW..+C
            tiles = []
            for ci in range(NCI):
                t = const.tile([P, C], BF16, name=f"{base_name}{ci}",
                               tag=f"{base_name}{ci}")
                r = m * WROW + ci * P
                (eng or nc.sync).dma_start(t, wg_d[r:r + P, :])
                tiles.append(t)
            return tiles

        # Input loads split between the SP queue and the (startup-idle) ACT
        # queue, ordered by first use. Dependent SBUF<->SBUF moves go on the
        # gpsimd queue so they can't FIFO-block behind these.
        # own q-slice Xt tiles come from the dequantized own slice via XBAR
        # transpose-DMA — no gather wait
        xtq_sb = []
        for ci in range(NCI):
            t = const.tile([P, SQ], BF16, name=f"xtqs{ci}", tag=f"xtqs{ci}")
            nc.scalar.dma_start(t, xnb_d[:, ci * P:(ci + 1) * P],
                                transpose=True)
            xtq_sb.append(t)
        wqt_sb = load_tiles("wqts", 0, eng=nc.scalar)
        wkt_sb = load_tiles("wkts", 1)
        xt_sb = [const.tile([P, S], BF16, name=f"xts{ci}", tag=f"xts{ci}")
                 for ci in range(NCI)]

        def load_xt_chunk(ci, ck):
            nc.sync.dma_start(
                xt_sb[ci][:, ck * SQC:(ck + 1) * SQC],
                xg_d[ck * SQC:(ck + 1) * SQC, ci * P:(ci + 1) * P],
                transpose=True)

        for ci in range(NCI):
            load_xt_chunk(ci, 0)
        wvt_sb = load_tiles("wvts", 2)
        for ck in range(1, S // SQC):
            for ci in range(NCI):
                load_xt_chunk(ci, ck)
        wot_sb = load_tiles("wots", 3)

        # bob = broadcast of bo (own input's last pack row, no gather
        # needed) to all 128 partitions via a K=1 ones-matmul
        bo1_sb = const.tile([1, C], BF16, name="bo1s", tag="bo1s")
        nc.sync.dma_start(bo1_sb, wsl_d[C:C + 1, :])
        one1_sb = const.tile([1, P], BF16, name="one1", tag="one1")
        nc.vector.memset(one1_sb, 1.0)
        bob_ps = psum.tile([P, C], F32, name="bobp", tag="proj")
        nc.tensor.matmul(bob_ps, lhsT=one1_sb, rhs=bo1_sb,
                         start=True, stop=True)
        bob_sb = const.tile([P, C], F32, name="bobs", tag="bobs")
        nc.vector.tensor_copy(out=bob_sb, in_=bob_ps)

        ones_sb = const.tile([P, D], mybir.dt.float16, name="ones_sb",
                             tag="ones_sb")
        nc.vector.memset(ones_sb, 1.0)

        for rep in range(reps):
            emit_body(nc, tc, const, work, psum, row_pack,
                      xt_sb, xtq_sb, wqt_sb, wkt_sb, wvt_sb, wot_sb,
                      bob_sb, ones_sb, outq_d, outs_d)

    nc.compile()
    return nc


def emit_body(nc, tc, const, work, psum, row_pack,
              xt_sb, xtq_sb, wqt_sb, wkt_sb, wvt_sb, wot_sb,
              bob_sb, ones_sb, outq_d, outs_d):
    vp_sb = [None] * NSK

    def emit_vproj(t_i):
        vps = psum.tile([P, C], F32, name="vps", tag="proj")
        for ci in range(NCI):
            nc.tensor.matmul(vps, lhsT=xt_sb[ci][:, t_i * P:(t_i + 1) * P],
                             rhs=wvt_sb[ci],
                             start=(ci == 0), stop=(ci == NCI - 1))
        vp = const.tile([P, H * E], BF16, name=f"vp{t_i}", tag=f"vp{t_i}")
        vp3 = vp.rearrange("p (h e) -> p h e", e=E)
        nc.vector.tensor_copy(out=vp3[:, :, 0:D],
                              in_=vps.rearrange("p (h d) -> p h d", d=D))
        nc.vector.memset(vp3[:, :, D:E], 1.0)
        vp_sb[t_i] = vp

    def emit_qtp(p):
        qtp = work.tile([P, SQ], BF16, name="qtp", tag="qtp")
        for cq in range(NSQC):
            qps = psum.tile([P, SQC], F32, name="qps", tag="proj")
            for ci in range(NCI):
                nc.tensor.matmul(
                    qps, lhsT=wqt_sb[ci][:, p * P:(p + 1) * P],
                    rhs=xtq_sb[ci][:, cq * SQC:(cq + 1) * SQC],
                    start=(ci == 0), stop=(ci == NCI - 1))
            nc.vector.tensor_copy(out=qtp[:, cq * SQC:(cq + 1) * SQC], in_=qps)
        return qtp

    def emit_ktp_chunk(ktp, p, ck):
        kps = psum.tile([P, SQC], F32, name="kps", tag="proj")
        for ci in range(NCI):
            nc.tensor.matmul(
                kps, lhsT=wkt_sb[ci][:, p * P:(p + 1) * P],
                rhs=xt_sb[ci][:, ck * SQC:(ck + 1) * SQC],
                start=(ci == 0), stop=(ci == NCI - 1))
        nc.vector.tensor_copy(out=ktp[:, ck * SQC:(ck + 1) * SQC], in_=kps)

    # Ot: normalized attention output, head-concat layout [c_in, sq]
    ot_sb = [const.tile([P, SQ], BF16, name=f"ot{i}", tag=f"ot{i}")
             for i in range(NCI)]

    def make_norm_tail(h, oraw, r):
        """Broadcast-matmul + normalize for head h. Deferred into the next
        head's loop so the PE-stream bcast matmul never waits on the DVE
        recip (PE is in-order; an early bcast would bubble the pipeline)."""
        def tail():
            rbp = psum.tile([D, SQ], F32, name="rbp", tag="st")
            for cq in range(NSQC):
                sl = slice(cq * SQC, (cq + 1) * SQC)
                nc.tensor.matmul(rbp[:, sl], lhsT=ones_sb[D:D + 1, :],
                                 rhs=r[D:D + 1, sl], start=True, stop=True)
            rb = work.tile([D, SQ], F32, name="rb", tag="rb", bufs=2)
            nc.vector.tensor_copy(out=rb, in_=rbp)
            if h % 2 == 0:
                nc.vector.tensor_mul(out=ot_sb[h // 2][0:D, :],
                                     in0=oraw[0:D, :], in1=rb)
            else:
                # DVE lanes are partition-locked; move to the upper half by DMA
                otmp = work.tile([D, SQ], BF16, name="otmp", tag="otmp",
                                 bufs=2)
                nc.vector.tensor_mul(out=otmp, in0=oraw[0:D, :], in1=rb)
                nc.gpsimd.dma_start(ot_sb[h // 2][D:2 * D, :], otmp)
        return tail

    outacc = const.tile([P, S], F16, name="outacc", tag="outacc")

    def make_oproj_tail(pair):
        """Accumulate pair `pair`'s output-projection contribution into
        outacc (SBUF). Deferred so only the final pair's slice is in the
        kernel tail."""
        def tail():
            for sqt in range(SQ // P):
                ops = psum.tile([P, C], F32, name="ops", tag="proj")
                nc.tensor.matmul(ops,
                                 lhsT=ot_sb[pair][:, sqt * P:(sqt + 1) * P],
                                 rhs=wot_sb[pair], start=True, stop=True)
                osl = outacc[:, sqt * C:(sqt + 1) * C]
                if pair == 0:
                    nc.vector.tensor_add(osl, ops, bob_sb)
                else:
                    nc.vector.tensor_add(osl, osl, ops)
            if pair == NCI - 1:
                # row-quantize to int8: per-partition absmax -> scale
                for sqt in range(SQ // P):
                    osl = outacc[:, sqt * C:(sqt + 1) * C]
                    mx = work.tile([P, 1], F32, name="omx", tag="omx", bufs=2)
                    nc.vector.tensor_reduce(
                        mx, osl, axis=mybir.AxisListType.X,
                        op=mybir.AluOpType.max, apply_absolute_value=True)
                    nc.vector.tensor_scalar_max(mx, mx, 1e-20)
                    ssl = work.tile([P, 1], F32, name="osc", tag="osc", bufs=2)
                    nc.vector.tensor_scalar_mul(ssl, mx, 1.0 / 127.0)
                    inv = work.tile([P, 1], F32, name="oinv", tag="oinv",
                                    bufs=2)
                    nc.vector.reciprocal(inv, ssl)
                    oq = work.tile([P, C], I8, name="oqt", tag="oqt", bufs=2)
                    nc.vector.tensor_scalar_mul(oq, osl, inv)
                    nc.gpsimd.dma_start(outq_d[sqt * P:(sqt + 1) * P, :], oq)
                    nc.gpsimd.dma_start(outs_d[sqt * P:(sqt + 1) * P, :], ssl)
        return tail

    ktp = qtp = None
    pending_norm = None
    pending_oproj = None
    next_pair = None          # (qtp, ktp, n_chunks_pre_emitted) for pair p+1
    pre_chunks = 0
    for h in range(H):
        p, half = h // 2, h % 2
        lo, hi = half * D, half * D + D          # head's rows in pair tiles
        olo, ohi = D - half * D, 2 * D - half * D  # opposite half rows

        if half == 0:
            if next_pair is not None:
                qtp, ktp, pre_chunks = next_pair
                next_pair = None
            else:
                qtp = emit_qtp(p)
                ktp = work.tile([P, S], BF16, name="ktp", tag="ktp")
                pre_chunks = 0
        # per-head swap copies: same rows duplicated into the other
        # partition half so both sq-chunks can use disjoint PE row groups
        if row_pack:
            dma_eng = nc.gpsimd
            qts = work.tile([P, SQ], BF16, name="qts", tag="qts")
            dma_eng.dma_start(qts[olo:ohi, :], qtp[lo:hi, :])
            kts = work.tile([P, S], BF16, name="kts", tag="kts")

        def emit_k_chunk(ck):
            if half == 0 and ck >= pre_chunks:
                emit_ktp_chunk(ktp, p, ck)
            if row_pack:
                dma_eng.dma_start(
                    kts[olo:ohi, ck * SQC:(ck + 1) * SQC],
                    ktp[lo:hi, ck * SQC:(ck + 1) * SQC])

        emit_k_chunk(0)
        oacc = psum.tile([E, SQ], F32, name="oacc", tag="oacc", bufs=1)
        for t_i in range(NSK):
            # prefetch the next K chunk one window early so the QK matmuls
            # never wait on the projection->evict->swap-DMA chain
            if t_i % 4 == 1 and t_i // 4 + 1 < S // SQC:
                emit_k_chunk(t_i // 4 + 1)
            if vp_sb[t_i] is None:
                emit_vproj(t_i)
            if t_i == 8 and pending_norm is not None:
                h_prev, tail = pending_norm
                tail()
                pending_norm = None
                if h_prev % 2 == 1:
                    pending_oproj = make_oproj_tail(h_prev // 2)
            if t_i == 16 and pending_oproj is not None:
                pending_oproj()
                pending_oproj = None
            # prefetch the next pair's Q/K projections late in the second
            # head of the current pair, so the pair boundary never stalls
            # ScalarE on the projection chain
            if t_i == 24 and half == 1 and h + 1 < H and next_pair is None:
                nq = emit_qtp(p + 1)
                nk = work.tile([P, S], BF16, name="ktp", tag="ktp")
                for ck0 in range(2):
                    emit_ktp_chunk(nk, p + 1, ck0)
                next_pair = (nq, nk, 2)

            st = psum.tile([P, SQ], F32, name="st", tag="st", bufs=2)
            ksl = slice(t_i * P, (t_i + 1) * P)
            if row_pack:
                nc.tensor.matmul(st[:, 0:SQC], lhsT=ktp[lo:hi, ksl],
                                 rhs=qtp[lo:hi, 0:SQC],
                                 start=True, stop=True,
                                 tile_position=(lo, 0))
                nc.tensor.matmul(st[:, SQC:SQ], lhsT=kts[olo:ohi, ksl],
                                 rhs=qts[olo:ohi, SQC:SQ],
                                 start=True, stop=True,
                                 tile_position=(olo, 0))
            else:
                for cq in range(NSQC):
                    nc.tensor.matmul(
                        st[:, cq * SQC:(cq + 1) * SQC],
                        lhsT=ktp[lo:hi, ksl],
                        rhs=qtp[lo:hi, cq * SQC:(cq + 1) * SQC],
                        start=True, stop=True)
            pt = work.tile([P, SQ], BF16, name="pt", tag="pt", bufs=3)
            nc.scalar.activation(out=pt, in_=st,
                                 func=mybir.ActivationFunctionType.Exp)
            for cq in range(NSQC):
                nc.tensor.matmul(
                    oacc[:, cq * SQC:(cq + 1) * SQC],
                    lhsT=vp_sb[t_i][:, h * E:(h + 1) * E],
                    rhs=pt[:, cq * SQC:(cq + 1) * SQC],
                    start=(t_i == 0), stop=(t_i == NSK - 1))

        # evict oacc to SBUF immediately so the PSUM slot frees for the next
        # head; the bcast+normalize runs deferred, off the critical path
        oraw = work.tile([E, SQ], F32, name="oraw", tag="oraw", bufs=2)
        nc.vector.tensor_copy(out=oraw, in_=oacc)
        r = work.tile([E, SQ], mybir.dt.float16, name="r", tag="r", bufs=2)
        with nc.allow_low_precision("softmax denom recip; fp16 ~1e-4 rel"):
            nc.vector.reciprocal(r[D:E, :], oraw[D:E, :])
        pending_norm = (h, make_norm_tail(h, oraw, r))

    if pending_oproj is not None:      # pair 2, if heads ended before t==16
        pending_oproj()
    pending_norm[1]()                  # final head's normalization
    make_oproj_tail(NCI - 1)()         # final pair's projection + store


def make_w_cat(Wq, Wk, Wv, Wo, bo):
    """Packed weight array [4*WROW, C] bf16, shared by both halves.
    Block m = [weight m transposed (q-scale folded for m=0); bo row]."""
    bf16 = ml_dtypes.bfloat16
    scale = np.float32(D) ** -0.5
    cat_w = np.empty((NHALF * WROW, C), bf16)
    bob = np.asarray(bo, np.float32)
    for m, w in enumerate((Wq, Wk, Wv, Wo)):
        blk = cat_w[m * WROW:(m + 1) * WROW]
        wt = np.asarray(w).T
        blk[0:C] = wt * scale if m == 0 else wt
        blk[C] = bob
    return cat_w


def make_x_cat(hidden_states, b):
    """int8 row-quantized X for batch b: ([S, C] int8, [S, 1] f32 scales).
    Core j's shard = rows [j*SQ, (j+1)*SQ)."""
    xb = np.asarray(hidden_states[b])
    m = np.abs(xb).max(axis=1, keepdims=True)
    s = np.maximum(m, np.float32(1e-20)) * np.float32(1 / 127.0)
    xq = np.rint(xb * (np.float32(1.0) / s)).astype(np.int8)
    return xq, s.astype(np.float32)


def _build_dispatch():
    """Compile the Bass module and build the cached jit dispatch closures."""
    import jax
    import jax.numpy as jnp
    from jax.sharding import Mesh, PartitionSpec, NamedSharding
    from jax.experimental.shard_map import shard_map
    from concourse.bass2jax import (
        _bass_exec_p, partition_id_tensor, install_neuronx_cc_hook)

    nc = build_nc()
    install_neuronx_cc_hook()
    assert nc.dbg_addr is None, "build with debug=False"

    partition_name = (nc.partition_id_tensor.name
                      if nc.partition_id_tensor else None)
    in_names, out_names, out_avals = [], [], []
    for alloc in nc.m.functions[0].allocations:
        if not isinstance(alloc, mybir.MemoryLocationSet):
            continue
        name = alloc.memorylocations[0].name
        if alloc.kind == "ExternalInput":
            if name != partition_name:
                in_names.append(name)
        elif alloc.kind == "ExternalOutput":
            shape = tuple(alloc.tensor_shape)
            dtype = mybir.dt.np(alloc.dtype)
            out_names.append(name)
            out_avals.append(jax.core.ShapedArray(shape, dtype))
    n_params = len(in_names)
    n_outs = len(out_avals)
    all_names = in_names + out_names + (
        [partition_name] if partition_name else [])

    def _body(*args):
        operands = list(args)
        if partition_name is not None:
            operands.append(partition_id_tensor())
        outs = _bass_exec_p.bind(
            *operands,
            out_avals=tuple(out_avals),
            in_names=tuple(all_names),
            out_names=tuple(out_names),
            lowering_input_output_aliases=(),
            sim_require_finite=True,
            sim_require_nnan=True,
            nc=nc,
        )
        return tuple(outs)

    devices = jax.devices()[:NCORES]
    halves = []
    for hi in range(2):
        mesh = Mesh(np.asarray(devices[hi * NHALF:(hi + 1) * NHALF]),
                    ("core",))
        in_specs = (PartitionSpec("core"),) * (n_params + n_outs)
        out_specs = (PartitionSpec("core"),) * n_outs
        # No donation: the kernel writes every element of `out`, so the
        # output operands are never read and can be device-resident dummies
        # reused across calls (zero wire traffic, zero per-call work).
        sharded = jax.jit(
            shard_map(_body, mesh=mesh, in_specs=in_specs,
                      out_specs=out_specs, check_rep=False),
            keep_unused=True)
        zero_shardings = tuple(
            NamedSharding(mesh, PartitionSpec("core")) for _ in range(n_outs))
        zeros = jax.jit(
            lambda: tuple(
                jnp.zeros((NHALF * a.shape[0], *a.shape[1:]), a.dtype)
                for a in out_avals),
            out_shardings=zero_shardings)()
        halves.append({"sharded": sharded, "zeros": zeros})

    return {
        "nc": nc,
        "halves": halves,
        "in_names": in_names,
        "outq_idx": out_names.index("outq"),
        "outs_idx": out_names.index("outs"),
        "pool": ThreadPoolExecutor(2),
    }


_CACHE = {}


def _get_dispatch():
    if "d" not in _CACHE:
        _CACHE["d"] = _build_dispatch()
    return _CACHE["d"]


def _pull(arrs):
    for a in arrs:
        for sh in a.addressable_shards:
            sh.data.copy_to_host_async()
    return [np.asarray(a) for a in arrs]


def run(inputs, trace=False, **kwargs):
    """Run on hardware; returns (full_output [B,S,C] fp32, aux)."""
    d = _get_dispatch()
    hs = inputs["hidden_states"]
    cat_w = make_w_cat(inputs["Wq"], inputs["Wk"], inputs["Wv"],
                       inputs["Wo"], inputs["bo"])
    by_name = {"wsl": cat_w}

    outs_j = []
    for hi in range(2):
        by_name["xq"], by_name["xs"] = make_x_cat(hs, hi)
        half = d["halves"][hi]
        arrs = half["sharded"](*[by_name[n] for n in d["in_names"]],
                               *half["zeros"])
        outs_j.append((arrs[d["outq_idx"]], arrs[d["outs_idx"]]))

    # pull batch 0 in a worker thread so its download overlaps batch 1's
    # upload + exec on the full-duplex tunnel
    fut0 = d["pool"].submit(_pull, outs_j[0])
    q1, s1 = _pull(outs_j[1])
    q0, s0 = fut0.result()

    out = np.empty((B, S, C), np.float32)
    np.multiply(q0.reshape(S, C), s0.reshape(S, 1), out=out[0],
                casting="unsafe")
    np.multiply(q1.reshape(S, C), s1.reshape(S, 1), out=out[1],
                casting="unsafe")
    return out, None


def kernel(**inputs):
    out, _ = run(inputs)
    return out


# revision 34
# speedup vs baseline: 8.6897x; 1.1418x over previous
"""Trainium2 Bass kernel for diffusers AttnProcessor self-attention.

Reference computation (fp32, B=2, S=4096, C=512, H=8, D=64):
    q = hs @ Wq.T ; k = hs @ Wk.T ; v = hs @ Wv.T          (per-head split)
    probs = softmax(q k^T / sqrt(D))                        [b,h,s,s]
    out = (probs @ v) @ Wo.T + bo                           [b,s,c]

Sharding: 8 cores = (batch b in 0..1) x (query-slice of 1024 rows in 0..3).
The two batch groups are compiled as INDEPENDENT 4-core launches (meshes
over devices 0-3 and 4-7) so the axon tunnel — which is full-duplex but
only ~40MB/s each way and dominates wall-clock — can overlap batch 0's
output download with batch 1's input upload.

Each core receives ONLY its own 1024-row Xt slice plus a 1/4 slice of the
packed projection weights (+ a bo row); device-side 4-way AllGathers
rebuild the full blocked Xt and weight pack, so the host->device wire
carries each input byte exactly once per half. Output returns as fp16.

Device dataflow per core (all matmuls bf16 in / fp32 PSUM accum):
  wg = AllGather(wsl)  [4*513, C]  (wqt|wkt|wvt|wot blocks, q-scale folded,
                                    one bo row per block)
  xg = AllGather(xtq)  [4C, SQ]    (blocked Xt: block qb = Xt[:, qb*SQ:..])
  Qt = wqt^T @ xtq  per head-pair                  [128, 1024]
  Kt = wkt^T @ xg   per head-pair                  [128, 4096]
  (a per-head copy of Qt/Kt rows is DMA'd to the opposite partition half so
   the two sq-chunks of the QK^T matmul run in disjoint PE row groups)
  V' = [xg @ wvt | 1] per head                     [S, 65] per head
  per head h, per key tile t (128 keys):
    St[t] = Kt_h[:,t]^T Qt_h        [128 sk, 1024 sq]  (2 row-packed matmuls)
    Pt    = exp(St)                 (ScalarE, bf16 out)
    O'_h += V'[t]^T Pt              [65, 1024]  (row 64 = softmax denominator)
  O_h = O'_h[0:64] * (1/O'_h[64])   -> Ot (head-concat layout)
  out = Ot^T @ wot + bob            -> DMA out  [1024, 512] fp16

Dispatch: module-cached jax.jit(shard_map) closures (one per half) over the
bass_exec custom call. Output operands are device-resident dummies reused
across calls (the kernel writes every element of `out`, so their contents
never matter and they never cross the wire).
"""

import numpy as np
import ml_dtypes
from contextlib import ExitStack
from concurrent.futures import ThreadPoolExecutor

import concourse.bass as bass
import concourse.bacc as bacc
import concourse.mybir as mybir
import concourse.tile as tile

BF16 = mybir.dt.bfloat16
F16 = mybir.dt.float16
F32 = mybir.dt.float32
I8 = mybir.dt.int8

B, S, C, H, D = 2, 4096, 512, 8, 64
NCORES = 8
NHALF = 4          # cores per batch group / per launch
SQ = 1024          # query rows per core
P = 128            # partitions
NSK = S // P       # 32 key tiles
NCI = C // P       # 4 contraction tiles for projections
SQC = 512          # matmul moving free dim
NSQC = SQ // SQC   # 2
E = D + 1          # V' columns per head (64 v cols + ones col)
WROW = C + 1       # weight-pack rows shipped per core (one weight + bo row)

ROW_PACK = True    # run the two K=64 QK^T matmuls in disjoint PE row groups


def build_nc(row_pack=ROW_PACK, reps=1):
    # num_devices=8 with BOTH groups listed: NRT resolves a device's
    # replica group by GLOBAL device id, so one NEFF serves both 4-device
    # launches (devices 0-3 -> group 0, devices 4-7 -> group 1).
    nc = bacc.Bacc("TRN2", target_bir_lowering=False, debug=False,
                   num_devices=NCORES)

    # X ships int8 row-quantized in NATURAL [s, c] layout (halves the upload
    # vs bf16; rel err ~8e-3 vs the 2e-2 budget). The device dequantizes to
    # bf16 (per-row scales, ~15us) and the XBAR DMA-transpose produces the
    # [c, s] SBUF tiles the matmuls need.
    xq_d = nc.dram_tensor("xq", [SQ, C], I8, kind="ExternalInput").ap()
    xs_d = nc.dram_tensor("xs", [SQ, 1], F32, kind="ExternalInput").ap()
    # weights ship int8 row-quantized too; the full scale vector for the
    # 4-block pack is replicated to every core (8KB) so tile dequant needs
    # no rank-dependent indexing
    wsl_d = nc.dram_tensor("wsl", [WROW, C], I8, kind="ExternalInput").ap()
    wsc_d = nc.dram_tensor("wsc", [4 * WROW, 1], F32,
                           kind="ExternalInput").ap()
    # output also ships int8 row-quantized (+ per-row f32 scales)
    outq_d = nc.dram_tensor("outq", [SQ, C], I8, kind="ExternalOutput").ap()
    outs_d = nc.dram_tensor("outs", [SQ, 1], F32, kind="ExternalOutput").ap()
    # 4-core groups don't support Shared collective outputs (needs >4)
    wg_d = nc.dram_tensor("wg", [NHALF * WROW, C], I8, kind="Internal").ap()
    xg_d = nc.dram_tensor("xg", [S, C], BF16, kind="Internal").ap()
    # dequantized own slice: collective input (collectives cannot read I/O
    # tensors) and XBAR source for the own-q-slice Xt tiles
    xnb_d = nc.dram_tensor("xnb", [SQ, C], BF16, kind="Internal").ap()
    wb_d = nc.dram_tensor("wb", [WROW, C], I8, kind="Internal").ap()

    with ExitStack() as ctx:
        tc = ctx.enter_context(tile.TileContext(nc))
        const = ctx.enter_context(tc.tile_pool(name="const", bufs=1))
        work = ctx.enter_context(tc.tile_pool(name="work", bufs=2))
        psum = ctx.enter_context(tc.tile_pool(name="psum", bufs=2, space="PSUM"))

        # Device-side gathers: weights first (first QK tile needs wqt+wkt),
        # then the batch group's Xt blocks. Collectives run on gpsimd in
        # issue order.
        groups = [[0, 1, 2, 3], [4, 5, 6, 7]]
        nc.gpsimd.dma_start(wb_d, wsl_d)
        # dequantize the own X slice to bf16 (per-row scale on partitions)
        for si in range(SQ // P):
            sl = slice(si * P, (si + 1) * P)
            xqt = work.tile([P, C], I8, name="xqt", tag="xqt", bufs=2)
            nc.sync.dma_start(xqt, xq_d[sl, :])
            xst = work.tile([P, 1], F32, name="xst", tag="xst", bufs=2)
            nc.sync.dma_start(xst, xs_d[sl, :])
            xbt = work.tile([P, C], BF16, name="xbt", tag="xbt", bufs=2)
            nc.vector.tensor_scalar_mul(xbt, xqt, xst)
            nc.gpsimd.dma_start(xnb_d[sl, :], xbt)
        nc.gpsimd.collective_compute(
            "AllGather", mybir.AluOpType.bypass,
            replica_groups=groups, ins=[wb_d], outs=[wg_d])
        nc.gpsimd.collective_compute(
            "AllGather", mybir.AluOpType.bypass,
            replica_groups=groups, ins=[xnb_d], outs=[xg_d])

        def load_tiles(base_name, m, eng=None):
            # weight m's [C, C] block lives at gathered rows m*WROW..+C;
            # dequantize to bf16 at load (scale is per pack row = partition)
            tiles = []
            for ci in range(NCI):
                r = m * WROW + ci * P
                tq = work.tile([P, C], I8, name=f"{base_name}q{ci}",
                               tag=f"{base_name}q{ci}")
                (eng or nc.sync).dma_start(tq, wg_d[r:r + P, :])
                ts = work.tile([P, 1], F32, name=f"{base_name}s{ci}",
                               tag=f"{base_name}s{ci}")
                (eng or nc.sync).dma_start(ts, wsc_d[r:r + P, :])
                t = const.tile([P, C], BF16, name=f"{base_name}{ci}",
                               tag=f"{base_name}{ci}")
                nc.vector.tensor_scalar_mul(t, tq, ts)
                tiles.append(t)
            return tiles

        # Input loads split between the SP queue and the (startup-idle) ACT
        # queue, ordered by first use. Dependent SBUF<->SBUF moves go on the
        # gpsimd queue so they can't FIFO-block behind these.
        # own q-slice Xt tiles come from the dequantized own slice via XBAR
        # transpose-DMA — no gather wait
        xtq_sb = []
        for ci in range(NCI):
            t = const.tile([P, SQ], BF16, name=f"xtqs{ci}", tag=f"xtqs{ci}")
            nc.scalar.dma_start(t, xnb_d[:, ci * P:(ci + 1) * P],
                                transpose=True)
            xtq_sb.append(t)
        wqt_sb = load_tiles("wqts", 0, eng=nc.scalar)
        wkt_sb = load_tiles("wkts", 1)
        xt_sb = [const.tile([P, S], BF16, name=f"xts{ci}", tag=f"xts{ci}")
                 for ci in range(NCI)]

        def load_xt_chunk(ci, ck):
            nc.sync.dma_start(
                xt_sb[ci][:, ck * SQC:(ck + 1) * SQC],
                xg_d[ck * SQC:(ck + 1) * SQC, ci * P:(ci + 1) * P],
                transpose=True)

        for ci in range(NCI):
            load_xt_chunk(ci, 0)
        wvt_sb = load_tiles("wvts", 2)
        for ck in range(1, S // SQC):
            for ci in range(NCI):
                load_xt_chunk(ci, ck)
        wot_sb = load_tiles("wots", 3)

        # bob = broadcast of bo (own input's last pack row, no gather
        # needed; identical on every core) to all 128 partitions via a K=1
        # ones-matmul
        bo1_q = const.tile([1, C], I8, name="bo1q", tag="bo1q")
        nc.sync.dma_start(bo1_q, wsl_d[C:C + 1, :])
        bo1_s = const.tile([1, 1], F32, name="bo1sc", tag="bo1sc")
        nc.sync.dma_start(bo1_s, wsc_d[C:C + 1, :])
        bo1_sb = const.tile([1, C], BF16, name="bo1s", tag="bo1s")
        nc.vector.tensor_scalar_mul(bo1_sb, bo1_q, bo1_s)
        one1_sb = const.tile([1, P], BF16, name="one1", tag="one1")
        nc.vector.memset(one1_sb, 1.0)
        bob_ps = psum.tile([P, C], F32, name="bobp", tag="proj")
        nc.tensor.matmul(bob_ps, lhsT=one1_sb, rhs=bo1_sb,
                         start=True, stop=True)
        bob_sb = const.tile([P, C], F32, name="bobs", tag="bobs")
        nc.vector.tensor_copy(out=bob_sb, in_=bob_ps)

        ones_sb = const.tile([P, D], mybir.dt.float16, name="ones_sb",
                             tag="ones_sb")
        nc.vector.memset(ones_sb, 1.0)

        for rep in range(reps):
            emit_body(nc, tc, const, work, psum, row_pack,
                      xt_sb, xtq_sb, wqt_sb, wkt_sb, wvt_sb, wot_sb,
                      bob_sb, ones_sb, outq_d, outs_d)

    nc.compile()
    return nc


def emit_body(nc, tc, const, work, psum, row_pack,
              xt_sb, xtq_sb, wqt_sb, wkt_sb, wvt_sb, wot_sb,
              bob_sb, ones_sb, outq_d, outs_d):
    vp_sb = [None] * NSK

    def emit_vproj(t_i):
        vps = psum.tile([P, C], F32, name="vps", tag="proj")
        for ci in range(NCI):
            nc.tensor.matmul(vps, lhsT=xt_sb[ci][:, t_i * P:(t_i + 1) * P],
                             rhs=wvt_sb[ci],
                             start=(ci == 0), stop=(ci == NCI - 1))
        vp = const.tile([P, H * E], BF16, name=f"vp{t_i}", tag=f"vp{t_i}")
        vp3 = vp.rearrange("p (h e) -> p h e", e=E)
        nc.vector.tensor_copy(out=vp3[:, :, 0:D],
                              in_=vps.rearrange("p (h d) -> p h d", d=D))
        nc.vector.memset(vp3[:, :, D:E], 1.0)
        vp_sb[t_i] = vp

    def emit_qtp(p):
        qtp = work.tile([P, SQ], BF16, name="qtp", tag="qtp")
        for cq in range(NSQC):
            qps = psum.tile([P, SQC], F32, name="qps", tag="proj")
            for ci in range(NCI):
                nc.tensor.matmul(
                    qps, lhsT=wqt_sb[ci][:, p * P:(p + 1) * P],
                    rhs=xtq_sb[ci][:, cq * SQC:(cq + 1) * SQC],
                    start=(ci == 0), stop=(ci == NCI - 1))
            nc.vector.tensor_copy(out=qtp[:, cq * SQC:(cq + 1) * SQC], in_=qps)
        return qtp

    def emit_ktp_chunk(ktp, p, ck):
        kps = psum.tile([P, SQC], F32, name="kps", tag="proj")
        for ci in range(NCI):
            nc.tensor.matmul(
                kps, lhsT=wkt_sb[ci][:, p * P:(p + 1) * P],
                rhs=xt_sb[ci][:, ck * SQC:(ck + 1) * SQC],
                start=(ci == 0), stop=(ci == NCI - 1))
        nc.vector.tensor_copy(out=ktp[:, ck * SQC:(ck + 1) * SQC], in_=kps)

    # Ot: normalized attention output, head-concat layout [c_in, sq]
    ot_sb = [const.tile([P, SQ], BF16, name=f"ot{i}", tag=f"ot{i}")
             for i in range(NCI)]

    def make_norm_tail(h, oraw, r):
        """Broadcast-matmul + normalize for head h. Deferred into the next
        head's loop so the PE-stream bcast matmul never waits on the DVE
        recip (PE is in-order; an early bcast would bubble the pipeline)."""
        def tail():
            rbp = psum.tile([D, SQ], F32, name="rbp", tag="st")
            for cq in range(NSQC):
                sl = slice(cq * SQC, (cq + 1) * SQC)
                nc.tensor.matmul(rbp[:, sl], lhsT=ones_sb[D:D + 1, :],
                                 rhs=r[D:D + 1, sl], start=True, stop=True)
            rb = work.tile([D, SQ], F32, name="rb", tag="rb", bufs=2)
            nc.vector.tensor_copy(out=rb, in_=rbp)
            if h % 2 == 0:
                nc.vector.tensor_mul(out=ot_sb[h // 2][0:D, :],
                                     in0=oraw[0:D, :], in1=rb)
            else:
                # DVE lanes are partition-locked; move to the upper half by DMA
                otmp = work.tile([D, SQ], BF16, name="otmp", tag="otmp",
                                 bufs=2)
                nc.vector.tensor_mul(out=otmp, in0=oraw[0:D, :], in1=rb)
                nc.gpsimd.dma_start(ot_sb[h // 2][D:2 * D, :], otmp)
        return tail

    outacc = const.tile([P, S], F16, name="outacc", tag="outacc")

    def make_oproj_tail(pair):
        """Accumulate pair `pair`'s output-projection contribution into
        outacc (SBUF). Deferred so only the final pair's slice is in the
        kernel tail."""
        def tail():
            for sqt in range(SQ // P):
                ops = psum.tile([P, C], F32, name="ops", tag="proj")
                nc.tensor.matmul(ops,
                                 lhsT=ot_sb[pair][:, sqt * P:(sqt + 1) * P],
                                 rhs=wot_sb[pair], start=True, stop=True)
                osl = outacc[:, sqt * C:(sqt + 1) * C]
                if pair == 0:
                    nc.vector.tensor_add(osl, ops, bob_sb)
                else:
                    nc.vector.tensor_add(osl, osl, ops)
            if pair == NCI - 1:
                # row-quantize to int8: per-partition absmax -> scale
                for sqt in range(SQ // P):
                    osl = outacc[:, sqt * C:(sqt + 1) * C]
                    mx = work.tile([P, 1], F32, name="omx", tag="omx", bufs=2)
                    nc.vector.tensor_reduce(
                        mx, osl, axis=mybir.AxisListType.X,
                        op=mybir.AluOpType.max, apply_absolute_value=True)
                    nc.vector.tensor_scalar_max(mx, mx, 1e-20)
                    ssl = work.tile([P, 1], F32, name="osc", tag="osc", bufs=2)
                    nc.vector.tensor_scalar_mul(ssl, mx, 1.0 / 127.0)
                    inv = work.tile([P, 1], F32, name="oinv", tag="oinv",
                                    bufs=2)
                    nc.vector.reciprocal(inv, ssl)
                    oq = work.tile([P, C], I8, name="oqt", tag="oqt", bufs=2)
                    nc.vector.tensor_scalar_mul(oq, osl, inv)
                    nc.gpsimd.dma_start(outq_d[sqt * P:(sqt + 1) * P, :], oq)
                    nc.gpsimd.dma_start(outs_d[sqt * P:(sqt + 1) * P, :], ssl)
        return tail

    ktp = qtp = None
    pending_norm = None
    pending_oproj = None
    next_pair = None          # (qtp, ktp, n_chunks_pre_emitted) for pair p+1
    pre_chunks = 0
    for h in range(H):
        p, half = h // 2, h % 2
        lo, hi = half * D, half * D + D          # head's rows in pair tiles
        olo, ohi = D - half * D, 2 * D - half * D  # opposite half rows

        if half == 0:
            if next_pair is not None:
                qtp, ktp, pre_chunks = next_pair
                next_pair = None
            else:
                qtp = emit_qtp(p)
                ktp = work.tile([P, S], BF16, name="ktp", tag="ktp")
                pre_chunks = 0
        # per-head swap copies: same rows duplicated into the other
        # partition half so both sq-chunks can use disjoint PE row groups
        if row_pack:
            dma_eng = nc.gpsimd
            qts = work.tile([P, SQ], BF16, name="qts", tag="qts")
            dma_eng.dma_start(qts[olo:ohi, :], qtp[lo:hi, :])
            kts = work.tile([P, S], BF16, name="kts", tag="kts")

        def emit_k_chunk(ck):
            if half == 0 and ck >= pre_chunks:
                emit_ktp_chunk(ktp, p, ck)
            if row_pack:
                dma_eng.dma_start(
                    kts[olo:ohi, ck * SQC:(ck + 1) * SQC],
                    ktp[lo:hi, ck * SQC:(ck + 1) * SQC])

        emit_k_chunk(0)
        oacc = psum.tile([E, SQ], F32, name="oacc", tag="oacc", bufs=1)
        for t_i in range(NSK):
            # prefetch the next K chunk one window early so the QK matmuls
            # never wait on the projection->evict->swap-DMA chain
            if t_i % 4 == 1 and t_i // 4 + 1 < S // SQC:
                emit_k_chunk(t_i // 4 + 1)
            if vp_sb[t_i] is None:
                emit_vproj(t_i)
            if t_i == 8 and pending_norm is not None:
                h_prev, tail = pending_norm
                tail()
                pending_norm = None
                if h_prev % 2 == 1:
                    pending_oproj = make_oproj_tail(h_prev // 2)
            if t_i == 16 and pending_oproj is not None:
                pending_oproj()
                pending_oproj = None
            # prefetch the next pair's Q/K projections late in the second
            # head of the current pair, so the pair boundary never stalls
            # ScalarE on the projection chain
            if t_i == 24 and half == 1 and h + 1 < H and next_pair is None:
                nq = emit_qtp(p + 1)
                nk = work.tile([P, S], BF16, name="ktp", tag="ktp")
                for ck0 in range(2):
                    emit_ktp_chunk(nk, p + 1, ck0)
                next_pair = (nq, nk, 2)

            st = psum.tile([P, SQ], F32, name="st", tag="st", bufs=2)
            ksl = slice(t_i * P, (t_i + 1) * P)
            if row_pack:
                nc.tensor.matmul(st[:, 0:SQC], lhsT=ktp[lo:hi, ksl],
                                 rhs=qtp[lo:hi, 0:SQC],
                                 start=True, stop=True,
                                 tile_position=(lo, 0))
                nc.tensor.matmul(st[:, SQC:SQ], lhsT=kts[olo:ohi, ksl],
                                 rhs=qts[olo:ohi, SQC:SQ],
                                 start=True, stop=True,
                                 tile_position=(olo, 0))
            else:
                for cq in range(NSQC):
                    nc.tensor.matmul(
                        st[:, cq * SQC:(cq + 1) * SQC],
                        lhsT=ktp[lo:hi, ksl],
                        rhs=qtp[lo:hi, cq * SQC:(cq + 1) * SQC],
                        start=True, stop=True)
            pt = work.tile([P, SQ], BF16, name="pt", tag="pt", bufs=3)
            nc.scalar.activation(out=pt, in_=st,
                                 func=mybir.ActivationFunctionType.Exp)
            for cq in range(NSQC):
                nc.tensor.matmul(
                    oacc[:, cq * SQC:(cq + 1) * SQC],
                    lhsT=vp_sb[t_i][:, h * E:(h + 1) * E],
                    rhs=pt[:, cq * SQC:(cq + 1) * SQC],
                    start=(t_i == 0), stop=(t_i == NSK - 1))

        # evict oacc to SBUF immediately so the PSUM slot frees for the next
        # head; the bcast+normalize runs deferred, off the critical path
        oraw = work.tile([E, SQ], F32, name="oraw", tag="oraw", bufs=2)
        nc.vector.tensor_copy(out=oraw, in_=oacc)
        r = work.tile([E, SQ], mybir.dt.float16, name="r", tag="r", bufs=2)
        with nc.allow_low_precision("softmax denom recip; fp16 ~1e-4 rel"):
            nc.vector.reciprocal(r[D:E, :], oraw[D:E, :])
        pending_norm = (h, make_norm_tail(h, oraw, r))

    if pending_oproj is not None:      # pair 2, if heads ended before t==16
        pending_oproj()
    pending_norm[1]()                  # final head's normalization
    make_oproj_tail(NCI - 1)()         # final pair's projection + store


def make_w_cat(Wq, Wk, Wv, Wo, bo):
    """int8 row-quantized weight pack ([4*WROW, C] int8, [4*WROW, 1] f32
    scales), shared by both halves. Block m = [weight m transposed (q-scale
    folded for m=0); bo row]."""
    scale = np.float32(D) ** -0.5
    cat_w = np.empty((NHALF * WROW, C), np.float32)
    bob = np.asarray(bo, np.float32)
    for m, w in enumerate((Wq, Wk, Wv, Wo)):
        blk = cat_w[m * WROW:(m + 1) * WROW]
        wt = np.asarray(w).T
        blk[0:C] = wt * scale if m == 0 else wt
        blk[C] = bob
    m_ = np.abs(cat_w).max(axis=1, keepdims=True)
    s = np.maximum(m_, np.float32(1e-20)) * np.float32(1 / 127.0)
    wq = np.rint(cat_w * (np.float32(1.0) / s)).astype(np.int8)
    return wq, s.astype(np.float32)


def make_x_cat(hidden_states, b):
    """int8 row-quantized X for batch b: ([S, C] int8, [S, 1] f32 scales).
    Core j's shard = rows [j*SQ, (j+1)*SQ)."""
    xb = np.asarray(hidden_states[b])
    m = np.abs(xb).max(axis=1, keepdims=True)
    s = np.maximum(m, np.float32(1e-20)) * np.float32(1 / 127.0)
    xq = np.rint(xb * (np.float32(1.0) / s)).astype(np.int8)
    return xq, s.astype(np.float32)


def _build_dispatch():
    """Compile the Bass module and build the cached jit dispatch closures."""
    import jax
    import jax.numpy as jnp
    from jax.sharding import Mesh, PartitionSpec, NamedSharding
    from jax.experimental.shard_map import shard_map
    from concourse.bass2jax import (
        _bass_exec_p, partition_id_tensor, install_neuronx_cc_hook)

    nc = build_nc()
    install_neuronx_cc_hook()
    assert nc.dbg_addr is None, "build with debug=False"

    partition_name = (nc.partition_id_tensor.name
                      if nc.partition_id_tensor else None)
    in_names, out_names, out_avals = [], [], []
    for alloc in nc.m.functions[0].allocations:
        if not isinstance(alloc, mybir.MemoryLocationSet):
            continue
        name = alloc.memorylocations[0].name
        if alloc.kind == "ExternalInput":
            if name != partition_name:
                in_names.append(name)
        elif alloc.kind == "ExternalOutput":
            shape = tuple(alloc.tensor_shape)
            dtype = mybir.dt.np(alloc.dtype)
            out_names.append(name)
            out_avals.append(jax.core.ShapedArray(shape, dtype))
    n_params = len(in_names)
    n_outs = len(out_avals)
    all_names = in_names + out_names + (
        [partition_name] if partition_name else [])

    def _body(*args):
        operands = list(args)
        if partition_name is not None:
            operands.append(partition_id_tensor())
        outs = _bass_exec_p.bind(
            *operands,
            out_avals=tuple(out_avals),
            in_names=tuple(all_names),
            out_names=tuple(out_names),
            lowering_input_output_aliases=(),
            sim_require_finite=True,
            sim_require_nnan=True,
            nc=nc,
        )
        return tuple(outs)

    devices = jax.devices()[:NCORES]
    halves = []
    for hi in range(2):
        mesh = Mesh(np.asarray(devices[hi * NHALF:(hi + 1) * NHALF]),
                    ("core",))
        in_specs = (PartitionSpec("core"),) * (n_params + n_outs)
        out_specs = (PartitionSpec("core"),) * n_outs
        # No donation: the kernel writes every element of `out`, so the
        # output operands are never read and can be device-resident dummies
        # reused across calls (zero wire traffic, zero per-call work).
        sharded = jax.jit(
            shard_map(_body, mesh=mesh, in_specs=in_specs,
                      out_specs=out_specs, check_rep=False),
            keep_unused=True)
        zero_shardings = tuple(
            NamedSharding(mesh, PartitionSpec("core")) for _ in range(n_outs))
        zeros = jax.jit(
            lambda: tuple(
                jnp.zeros((NHALF * a.shape[0], *a.shape[1:]), a.dtype)
                for a in out_avals),
            out_shardings=zero_shardings)()
        halves.append({"sharded": sharded, "zeros": zeros})

    return {
        "nc": nc,
        "halves": halves,
        "in_names": in_names,
        "outq_idx": out_names.index("outq"),
        "outs_idx": out_names.index("outs"),
        "pool": ThreadPoolExecutor(2),
    }


_CACHE = {}


def _get_dispatch():
    if "d" not in _CACHE:
        _CACHE["d"] = _build_dispatch()
    return _CACHE["d"]


def _pull(arrs):
    for a in arrs:
        for sh in a.addressable_shards:
            sh.data.copy_to_host_async()
    return [np.asarray(a) for a in arrs]


def run(inputs, trace=False, **kwargs):
    """Run on hardware; returns (full_output [B,S,C] fp32, aux)."""
    d = _get_dispatch()
    hs = inputs["hidden_states"]
    cat_w, w_scales = make_w_cat(inputs["Wq"], inputs["Wk"], inputs["Wv"],
                                 inputs["Wo"], inputs["bo"])
    by_name = {"wsl": cat_w, "wsc": np.tile(w_scales, (NHALF, 1))}

    outs_j = []
    for hi in range(2):
        by_name["xq"], by_name["xs"] = make_x_cat(hs, hi)
        half = d["halves"][hi]
        arrs = half["sharded"](*[by_name[n] for n in d["in_names"]],
                               *half["zeros"])
        outs_j.append((arrs[d["outq_idx"]], arrs[d["outs_idx"]]))

    # pull batch 0 in a worker thread so its download overlaps batch 1's
    # upload + exec on the full-duplex tunnel
    fut0 = d["pool"].submit(_pull, outs_j[0])
    q1, s1 = _pull(outs_j[1])
    q0, s0 = fut0.result()

    out = np.empty((B, S, C), np.float32)
    np.multiply(q0.reshape(S, C), s0.reshape(S, 1), out=out[0],
                casting="unsafe")
    np.multiply(q1.reshape(S, C), s1.reshape(S, 1), out=out[1],
                casting="unsafe")
    return out, None


def kernel(**inputs):
    out, _ = run(inputs)
    return out
